# revision 28
# baseline (speedup 1.0000x reference)
"""Trainium2 Bass kernel for a dense transformer block (LN1 -> MHA -> LN2 -> MLP).

Sharding: 8 cores = (batch b in 0..3) x (sequence half in 0..1), zero
cross-core communication. Each core's input tokens are reordered on the host
so its 1024 query tokens are always tokens 0..1023 of its 2048-token view
(key/value order is irrelevant to attention), letting one SPMD program serve
every core and the query-side LN reuse the full-sequence LN output.

Precision: fp8e4m3 DoubleRow matmuls for QKV/O projections, ctx, and the MLP
(weights pre-scaled by power-of-2 factors on the host; descales fold into
existing bias/scale stages, so they cost nothing). Scores stay bf16.
LayerNorm gain/bias are folded into the following weights on the host
(mathematically exact), so the device LN is a pure (x-mu)*rstd normalize.

Softmax: exp(score - C) with a host-estimated shift C keeping exp outputs in
fp8 range; the denominator is produced by a ones-column appended to V inside
the ctx DoubleRow matmul (out partition 65), so it costs no extra PE time.

Schedule: the Act engine's exp stream (16 blocks x ~16us) is the backbone;
everything else is emitted as "filler units" pulled into slots between the
per-kcp score/ctx groups of each attention block so the in-order PE queue
stays fed without ever delaying the next scores (which would starve Act).
Projections stream into the qq=0 attention blocks; wo/ln2/fc1 of qq=0 stream
into the qq=1 blocks. fc1 results are staged pre-gelu in bf16 (aliasing the
Y1 arena via bitcast) so gelus run as two contiguous islands (2 activation-
table switches instead of ~64); fc2(0) fills the late qq=1 blocks. Weights
are pre-swizzled on the host into the exact SBUF tile images so every weight
DMA is fully contiguous (2x descriptor throughput vs 128B strides).
"""

import math
import sys

if '/opt/trn_rl_repo' not in sys.path:
    sys.path.insert(0, '/opt/trn_rl_repo')

import numpy as np
import ml_dtypes

import concourse.tile as tile
import concourse.mybir as mybir
from concourse import bacc
from concourse.bass import ts
from concourse.bass_utils import run_bass_kernel_spmd

P = 128
F32 = mybir.dt.float32
F32R = mybir.dt.float32r
BF16 = mybir.dt.bfloat16
F8 = mybir.dt.float8e4
AF = mybir.ActivationFunctionType
DR = mybir.MatmulPerfMode.DoubleRow
ALU = mybir.AluOpType
EPS = 1e-6

B, S, D, H, MLP = 4, 2048, 1024, 16, 4096
N_CORES = 8
NP_F8 = ml_dtypes.float8_e4m3


def build_bass(T, Q, Dm, Hh, Mlp, n_cores, scales, dbg=False):
    s_wq, s_wk, s_wv, s_wo, s_w1, s_w2, shift_c = scales
    dh = Dm // Hh
    assert dh == 64
    n_dc = Dm // P          # 8 feature chunks
    n_cj = n_dc // 2        # 4 DoubleRow k-pair steps over D
    n_tk = T // P           # 16 token chunks
    TB = 256                # LN1 sub-block (keeps p_x/p_t small)
    n_tb = T // TB          # 8
    KTB = 512               # K projection token slice
    n_ktb = T // KTB        # 4
    QQ = 512
    n_qq = Q // QQ          # 2
    n_mo = Mlp // P         # 32
    n_m2 = n_mo // 2        # 16 DoubleRow k-pair steps over MLP
    n_hp = Hh // 2          # 8 head pairs
    NO = 512
    inv_d = 1.0 / Dm
    exp_scale = 0.125 / (s_wq * s_wk)
    c_wo = 1.0 / (s_wo * s_wv)
    inv_s1 = 1.0 / s_w1
    inv_s2 = 1.0 / s_w2

    nc = bacc.Bacc("TRN2", target_bir_lowering=False, debug=False,
                   enable_asserts=False, num_devices=n_cores)

    def din(name, shape, dt):
        return nc.dram_tensor(name, shape, dt, kind="ExternalInput").ap()

    xT_d = din("xT", (Dm, T), F32)
    # host-swizzled weight images: per-partition-contiguous SBUF tile layouts
    wq_d = din("wq8", (P, n_dc * n_cj * 2 * P), F8)
    wk_d = din("wk8", (P, n_dc * n_cj * 2 * P), F8)
    wv_d = din("wv8", (P, 2 * n_cj * 2 * NO), F8)
    wo_d = din("wo8", (2, P, n_cj * 2 * 4 * P), F8)
    w1_d = din("w18", (n_mo // 4, P, n_cj * 2 * 4 * P), F8)
    w2_d = din("w28", (2, n_dc, P, n_m2 * 2 * P), F8)
    bq_d, bk_d = din("bq", (Dm,), F32), din("bk", (Dm,), F32)
    bv_d, bo_d = din("bv16", (Dm,), BF16), din("bo", (Dm,), F32)
    b1_d, b2_d = din("b1", (Mlp,), F32), din("b2", (Dm,), F32)
    ones_d = din("ones32", (P, P), F32)
    yT_d = nc.dram_tensor("yT", (Dm, Q), F32, kind="ExternalOutput").ap()

    with tile.TileContext(nc) as tc, \
         tc.tile_pool(name="const", bufs=1) as constp, \
         tc.tile_pool(name="p_res", bufs=1) as p_res, \
         tc.tile_pool(name="p_kv", bufs=1) as p_kv, \
         tc.tile_pool(name="p_exp", bufs=2) as p_exp, \
         tc.tile_pool(name="p_rb", bufs=1) as p_rb, \
         tc.tile_pool(name="p_ct", bufs=1) as p_ct, \
         tc.tile_pool(name="p_wos", bufs=2) as p_wos, \
         tc.tile_pool(name="p_w1", bufs=2) as p_w1, \
         tc.tile_pool(name="p_out", bufs=1) as p_out, \
         tc.tile_pool(name="ps_sc", bufs=2, space="PSUM") as ps_sc, \
         tc.tile_pool(name="ps_ctx", bufs=2, space="PSUM") as ps_ctx, \
         tc.tile_pool(name="ps_sh", bufs=2, space="PSUM") as ps_sh:

        ones_fr = constp.tile([P, P], F32R)
        nc.sync.dma_start(ones_fr[:], ones_d[:, :].bitcast(F32R))
        ones_f = constp.tile([P, P], BF16)
        nc.vector.memset(ones_f[:], 1.0)
        negc_t = constp.tile([P, 1], F32)
        nc.vector.memset(negc_t[:], -shift_c)

        def vec_tile(src, n, nm):
            t = constp.tile([P, n], F32, tag=nm, name=nm)
            nc.sync.dma_start(t[:], src.rearrange("(c p) -> p c", p=P))
            return t

        bq_t, bk_t = vec_tile(bq_d, n_dc, "bq"), vec_tile(bk_d, n_dc, "bk")
        bo_t, b2_t = vec_tile(bo_d, n_dc, "bo"), vec_tile(b2_d, n_dc, "b2")
        b1_t = vec_tile(b1_d, n_mo, "b1")

        XQ = p_res.tile([P, n_dc, Q], BF16)       # residual stream (bf16)
        KT = p_kv.tile([P, n_dc, T], BF16)
        QT = p_kv.tile([P, n_dc, Q], BF16)
        VT = p_kv.tile([P, n_tk, Hh, 65], F8)
        nc.gpsimd.memset(VT[:, :, :, 64:65], 1.0)
        CT8 = p_ct.tile([P, n_dc, Q], F8)

        def attn_block(qq, hp, pull=None, fsc=1, fctx=1):
            """One head-pair of attention for query chunk qq, software-
            pipelined, with filler slots after each score/ctx group."""
            qsl = ts(qq, QQ)
            exps = [p_exp.tile([P, n_tk, QQ], F8, tag="expT", name="expT")
                    for _ in range(2)]
            pcs = [ps_ctx.tile([65, QQ], F32, tag="ps_c", name="ps_c")
                   for _ in range(2)]
            nk2 = n_tk // 2
            LAG = 2
            for j in range(nk2 + LAG):
                if j < nk2:
                    for hi in range(2):
                        r0 = hi * 64
                        pss = ps_sc.tile([P, 2, QQ], F32, tag="ps_s",
                                         name="ps_s")
                        for jj in range(2):
                            nc.tensor.matmul(
                                pss[:, jj, :],
                                KT[r0:r0 + 64, hp, ts(2 * j + jj, P)],
                                QT[r0:r0 + 64, hp, qsl],
                                start=True, stop=True)
                        nc.scalar.activation(
                            exps[hi][:, 2 * j:2 * j + 2, :],
                            pss[:, :, :], AF.Exp,
                            scale=exp_scale, bias=negc_t[:, 0:1])
                    if pull:
                        pull(fsc)
                if j >= LAG:
                    kcp = j - LAG
                    for hi in range(2):
                        h = 2 * hp + hi
                        nc.tensor.matmul(
                            pcs[hi][:, :],
                            VT[:, 2 * kcp:2 * kcp + 2, h, 0:65],
                            exps[hi][:, 2 * kcp:2 * kcp + 2, :],
                            start=(kcp == 0), stop=(kcp == nk2 - 1),
                            perf_mode=DR)
                    if pull:
                        pull(fctx)
            for hi in range(2):
                r0 = hi * 64
                rb1 = p_rb.tile([1, QQ], F32, tag="rb1")
                nc.vector.reciprocal(rb1[:], pcs[hi][64:65, :])
                rbb = p_rb.tile([64, QQ], F32, tag="rbb")
                nc.gpsimd.partition_broadcast(rbb[:], rb1[:])
                nc.vector.tensor_mul(CT8[r0:r0 + 64, hp, qsl],
                                     pcs[hi][0:64, :], rbb[:])

        def make_pull(gen):
            done = [False]
            def pull(n=1):
                if done[0]:
                    return
                for _ in range(n):
                    try:
                        next(gen)
                    except StopIteration:
                        done[0] = True
                        return
            return pull

        w1_map = {}

        def w1_need(g):
            if g not in w1_map:
                wt = p_w1.tile([P, n_cj, 2, 4, P], F8, tag="w1")
                nc.sync.dma_start(
                    wt[:].rearrange("p a b c d -> p (a b c d)"), w1_d[g])
                w1_map[g] = wt
            return w1_map[g]

        # ================= Phase A: LN1 + projections + attn(qq=0) =========
        with tc.tile_pool(name="p_xn8", bufs=1) as p_xn8, \
             tc.tile_pool(name="p_x", bufs=2) as p_x, \
             tc.tile_pool(name="p_t", bufs=2) as p_t, \
             tc.tile_pool(name="p_wk", bufs=1) as p_wk:

            XN8 = p_xn8.tile([P, n_dc, T], F8)
            bv_row = p_xn8.tile([1, Dm], BF16)
            nc.sync.dma_start(bv_row[:, :], bv_d[None, :])
            bv_bc = p_xn8.tile([P, Dm], BF16)
            nc.gpsimd.partition_broadcast(bv_bc[:], bv_row[:])

            wk_t = p_wk.tile([P, n_dc, n_cj, 2, P], F8, tag="wk")
            nc.sync.dma_start(
                wk_t[:].rearrange("p a b c d -> p (a b c d)"), wk_d[:, :])
            wq_t = p_wk.tile([P, n_dc, n_cj, 2, P], F8, tag="wq")
            nc.sync.dma_start(
                wq_t[:].rearrange("p a b c d -> p (a b c d)"), wq_d[:, :])
            wv_t = p_wk.tile([P, 2, n_cj, 2, NO], F8, tag="wv")
            nc.sync.dma_start(
                wv_t[:].rearrange("p a b c d -> p (a b c d)"), wv_d[:, :])

            def k_unit(mo, tb):
                tsl = ts(tb, KTB)
                ps = ps_sh.tile([P, KTB], F32, tag="sh")
                for cj in range(n_cj):
                    nc.tensor.matmul(
                        ps[:], wk_t[:, mo, cj, :, :],
                        XN8[:, 2 * cj:2 * cj + 2, tsl],
                        start=(cj == 0), stop=(cj == n_cj - 1),
                        perf_mode=DR)
                nc.vector.tensor_scalar_add(KT[:, mo, tsl], ps[:],
                                            bk_t[:, mo:mo + 1])

            def q_unit(mo, qb):
                qsl = ts(qb, QQ)
                ps = ps_sh.tile([P, QQ], F32, tag="sh")
                for cj in range(n_cj):
                    nc.tensor.matmul(
                        ps[:], wq_t[:, mo, cj, :, :],
                        XN8[:, 2 * cj:2 * cj + 2, qsl],
                        start=(cj == 0), stop=(cj == n_cj - 1),
                        perf_mode=DR)
                nc.vector.tensor_scalar_add(QT[:, mo, qsl], ps[:],
                                            bq_t[:, mo:mo + 1])

            def v_unit(no, to):
                ps = ps_sh.tile([P, NO], F32, tag="sh")
                for cj in range(n_cj):
                    nc.tensor.matmul(
                        ps[:], XN8[:, 2 * cj:2 * cj + 2, ts(to, P)],
                        wv_t[:, no, cj, :, :],
                        start=(cj == 0), stop=(cj == n_cj - 1),
                        perf_mode=DR)
                nc.vector.tensor_add(VT[:, to, 8 * no:8 * no + 8, 0:64],
                                     ps[:], bv_bc[:, ts(no, NO)])

            # ---- LN1 per token block; first projections ride along ----
            for tb in range(n_tb):
                tsl = ts(tb, TB)
                xc = p_x.tile([P, n_dc, TB], F32R, tag="xc")
                for dc in range(n_dc):
                    nc.sync.dma_start(xc[:, dc, :],
                                      xT_d[ts(dc, P), tsl].bitcast(F32R))
                st = ps_sc.tile([P, 2, TB], F32, tag="ps_s", name="ps_s")
                for dc in range(n_dc):
                    nc.tensor.matmul(st[:, 0, :], ones_fr[:], xc[:, dc, :],
                                     start=(dc == 0), stop=(dc == n_dc - 1))
                    xsq = p_t.tile([P, TB], BF16, tag="xsq")
                    nc.scalar.activation(xsq[:], xc[:, dc, :].bitcast(F32),
                                         AF.Square)
                    nc.tensor.matmul(st[:, 1, :], ones_f[:], xsq[:],
                                     start=(dc == 0), stop=(dc == n_dc - 1))
                    if tb < Q // TB:
                        nc.vector.tensor_copy(XQ[:, dc, tsl],
                                              xc[:, dc, :].bitcast(F32))
                mbc = p_t.tile([P, TB], F32, tag="mbc")
                nc.vector.tensor_scalar_mul(mbc[:], st[:, 0, :], inv_d)
                var = p_t.tile([P, TB], F32, tag="var")
                nc.vector.tensor_scalar(var[:], st[:, 1, :], inv_d, EPS,
                                        op0=ALU.mult, op1=ALU.add)
                m2 = p_t.tile([P, TB], F32, tag="tn")
                nc.vector.tensor_mul(m2[:], mbc[:], mbc[:])
                nc.vector.tensor_sub(var[:], var[:], m2[:])
                std = p_t.tile([P, TB], F32, tag="stdt")
                nc.scalar.activation(std[:], var[:], AF.Sqrt)
                rstd = p_t.tile([P, TB], F32, tag="rstd")
                nc.vector.reciprocal(rstd[:], std[:])
                for dc in range(n_dc):
                    t0 = p_t.tile([P, TB], F32, tag="tn")
                    nc.vector.tensor_sub(t0[:], xc[:, dc, :].bitcast(F32),
                                         mbc[:])
                    nc.gpsimd.tensor_mul(XN8[:, dc, tsl], t0[:], rstd[:])
                # K head-pair 0 + first V chunks ride each LN1 block
                for to in (2 * tb, 2 * tb + 1):
                    v_unit(0, to)
                if tb % 2 == 1:
                    k_unit(0, tb // 2)
            q_unit(0, 0)

            def projection_gen():
                # one ~0.4-0.9us unit per yield; ordered so k(hp)/q(hp,0)
                # land before attn(0,hp) and v(no1) before ctx of attn(0,4)
                for mo in range(1, 5):
                    for tb in range(n_ktb):
                        k_unit(mo, tb)
                        yield
                    q_unit(mo, 0)
                    yield
                for to in range(8):
                    v_unit(1, to)
                    yield
                for tb in range(n_ktb):
                    k_unit(5, tb)
                    yield
                q_unit(5, 0)
                yield
                for to in range(8, 16):
                    v_unit(1, to)
                    yield
                for mo in (6, 7):
                    for tb in range(n_ktb):
                        k_unit(mo, tb)
                        yield
                    q_unit(mo, 0)
                    yield
                for mo in range(n_dc):
                    q_unit(mo, 1)
                    yield
                # prefetch wo groups + w1 group 0 for phase B
                for g in range(2):
                    wt = p_wos.tile([P, n_cj, 2, 4, P], F8, tag="wo")
                    nc.sync.dma_start(
                        wt[:].rearrange("p a b c d -> p (a b c d)"),
                        wo_d[g])
                    wo_tiles.append(wt)
                yield
                w1_need(0)
                yield

            wo_tiles = []
            gA = projection_gen()
            pullA = make_pull(gA)
            for hp in range(n_hp):
                attn_block(0, hp, pull=pullA, fsc=1, fctx=0)
            for _ in gA:
                pass

        # ================= Phase B: attn(qq=1) + MLP =======================
        with tc.tile_pool(name="p_mlp", bufs=1) as p_mlp, \
             tc.tile_pool(name="p_t2", bufs=2) as p_t2, \
             tc.tile_pool(name="p_t2s", bufs=1) as p_t2s, \
             tc.tile_pool(name="p_w2", bufs=2) as p_w2, \
             tc.tile_pool(name="p_y1", bufs=1) as p_y1:

            XN2 = p_mlp.tile([P, n_dc, 2, Q], F8)  # [.., hi/lo, ..]
            y1s = {}

            def wo_unit(qq, mo):
                qsl = ts(qq, QQ)
                ps = ps_sh.tile([P, QQ], F32, tag="sh")
                for cj in range(n_cj):
                    nc.tensor.matmul(
                        ps[:], wo_tiles[mo // 4][:, cj, :, mo % 4, :],
                        CT8[:, 2 * cj:2 * cj + 2, qsl],
                        start=(cj == 0), stop=(cj == n_cj - 1),
                        perf_mode=DR)
                tw = p_out.tile([P, QQ], F32, tag="ot")
                nc.vector.tensor_scalar(tw[:], ps[:], c_wo,
                                        bo_t[:, mo:mo + 1],
                                        op0=ALU.mult, op1=ALU.add)
                nc.vector.tensor_add(XQ[:, mo, qsl], tw[:], XQ[:, mo, qsl])

            def ln2_block(qq):
                qsl = ts(qq, QQ)
                st2 = ps_sc.tile([P, 2, QQ], F32, tag="ps_s", name="ps_s")
                for dc in range(n_dc):
                    nc.tensor.matmul(st2[:, 0, :], ones_f[:], XQ[:, dc, qsl],
                                     start=(dc == 0), stop=(dc == n_dc - 1))
                    sq = p_t2.tile([P, QQ], BF16, tag="sq2")
                    nc.gpsimd.tensor_mul(sq[:], XQ[:, dc, qsl],
                                         XQ[:, dc, qsl])
                    nc.tensor.matmul(st2[:, 1, :], ones_f[:], sq[:],
                                     start=(dc == 0), stop=(dc == n_dc - 1))
                mbc = p_t2s.tile([P, QQ], F32, tag="mbc2")
                nc.vector.tensor_scalar_mul(mbc[:], st2[:, 0, :], inv_d)
                var = p_t2s.tile([P, QQ], F32, tag="var2")
                nc.vector.tensor_scalar(var[:], st2[:, 1, :], inv_d, EPS,
                                        op0=ALU.mult, op1=ALU.add)
                m2 = p_t2.tile([P, QQ], F32, tag="tn2")
                nc.vector.tensor_mul(m2[:], mbc[:], mbc[:])
                nc.vector.tensor_sub(var[:], var[:], m2[:])
                # rstd = rsqrt(var) on DVE only: seed from 1/var + Newton
                r = p_t2s.tile([P, QQ], F32, tag="rstd2")
                nc.vector.reciprocal_approx_fast(r[:], var[:])
                nc.vector.tensor_scalar(r[:], r[:], 0.72, 0.35,
                                        op0=ALU.mult, op1=ALU.add)
                for _ in range(3):
                    t1 = p_t2.tile([P, QQ], F32, tag="tn2")
                    nc.vector.tensor_mul(t1[:], r[:], r[:])
                    nc.vector.tensor_mul(t1[:], t1[:], var[:])
                    nc.vector.tensor_scalar(t1[:], t1[:], -0.5, 1.5,
                                            op0=ALU.mult, op1=ALU.add)
                    nc.vector.tensor_mul(r[:], r[:], t1[:])
                for dc in range(n_dc):
                    t0 = p_t2.tile([P, QQ], F32, tag="tn2")
                    nc.gpsimd.tensor_sub(t0[:], XQ[:, dc, qsl], mbc[:])
                    m = p_t2.tile([P, QQ], F32, tag="m32")
                    nc.vector.tensor_mul(m[:], t0[:], r[:])
                    nc.vector.tensor_copy(XN2[:, dc, 0, qsl], m[:])
                    nc.gpsimd.tensor_sub(XN2[:, dc, 1, qsl], m[:],
                                         XN2[:, dc, 0, qsl])

            def y1_tile(qq):
                if qq not in y1s:
                    y1s[qq] = p_y1.tile([P, n_mo, 2, QQ], F8, tag="y1",
                                        name="y1")
                return y1s[qq]

            def z1_view(Y1, mo):
                # bf16 view of Y1[:, mo, :, :]'s bytes (pre-gelu staging)
                return Y1[:, mo, :, :].bitcast(BF16).rearrange(
                    "p a b -> p (a b)")

            def fc1_mm_unit(qq, mo, staged):
                qsl = ts(qq, QQ)
                Y1 = y1_tile(qq)
                wt = w1_need(mo // 4)
                if mo % 4 == 0 and (mo // 4) + 1 < n_mo // 4:
                    w1_need((mo // 4) + 1)   # prefetch next group
                ps = ps_sh.tile([P, QQ], F32, tag="sh")
                for lv in range(2):
                    for cj in range(n_cj):
                        nc.tensor.matmul(
                            ps[:], wt[:, cj, :, mo % 4, :],
                            XN2[:, 2 * cj:2 * cj + 2, lv, qsl],
                            start=(lv == 0 and cj == 0),
                            stop=(lv == 1 and cj == n_cj - 1),
                            perf_mode=DR)
                if staged:
                    nc.vector.tensor_copy(z1_view(Y1, mo), ps[:])
                    return None
                return ps

            def gelu_island(qq, mo0, mo1):
                Y1 = y1_tile(qq)
                for mo in range(mo0, mo1):
                    g32 = p_t2.tile([P, QQ], F32, tag="m32")
                    nc.scalar.activation(g32[:], z1_view(Y1, mo), AF.Gelu,
                                         bias=b1_t[:, mo:mo + 1],
                                         scale=inv_s1)
                    nc.vector.tensor_copy(Y1[:, mo, 0, :], g32[:])
                    nc.gpsimd.tensor_sub(Y1[:, mo, 1, :], g32[:],
                                         Y1[:, mo, 0, :])

            def fc1_plain_unit(qq, mo):
                # tail variant: inline gelu (gelus end up consecutive)
                Y1 = y1_tile(qq)
                ps = fc1_mm_unit(qq, mo, staged=False)
                g32 = p_t2.tile([P, QQ], F32, tag="m32")
                nc.scalar.activation(g32[:], ps[:], AF.Gelu,
                                     bias=b1_t[:, mo:mo + 1],
                                     scale=inv_s1)
                nc.vector.tensor_copy(Y1[:, mo, 0, :], g32[:])
                nc.gpsimd.tensor_sub(Y1[:, mo, 1, :], g32[:],
                                     Y1[:, mo, 0, :])

            def fc2_gen(qq, m0=0, m1=None):
                qsl = ts(qq, QQ)
                if m1 is None:
                    m1 = n_dc
                Y1 = y1s[qq]
                if m1 == n_dc:
                    y1s.pop(qq)
                for mo2 in range(m0, m1):
                    wth = p_w2.tile([P, n_m2, 2, P], F8, tag="w2h")
                    nc.sync.dma_start(
                        wth[:].rearrange("p a b c -> p (a b c)"),
                        w2_d[0, mo2])
                    wtl = p_w2.tile([P, n_m2, 2, P], F8, tag="w2l")
                    nc.sync.dma_start(
                        wtl[:].rearrange("p a b c -> p (a b c)"),
                        w2_d[1, mo2])
                    ps = ps_sh.tile([P, QQ], F32, tag="sh")
                    terms = [(wth, 0), (wth, 1), (wtl, 0)]
                    for ti, (wt, lv) in enumerate(terms):
                        for cj in range(n_m2):
                            nc.tensor.matmul(
                                ps[:], wt[:, cj, :, :],
                                Y1[:, 2 * cj:2 * cj + 2, lv, :],
                                start=(ti == 0 and cj == 0),
                                stop=(ti == 2 and cj == n_m2 - 1),
                                perf_mode=DR)
                        yield
                    ot = p_out.tile([P, QQ], F32, tag="ot")
                    nc.vector.tensor_scalar(ot[:], ps[:], inv_s2,
                                            b2_t[:, mo2:mo2 + 1],
                                            op0=ALU.mult, op1=ALU.add)
                    nc.vector.tensor_add(ot[:], ot[:], XQ[:, mo2, qsl])
                    nc.sync.dma_start(yT_d[ts(mo2, P), qsl], ot[:])

            def mlp0_gen():
                # stretch fillers for qq=0 MLP, pulled into attn(1,*) slots
                for mo in range(n_dc):
                    wo_unit(0, mo)
                    yield
                ln2_block(0)
                # bubbles: let the DVE/Pool XN2 chain finish before the
                # first fc1 matmul hits the in-order PE queue
                for _ in range(12):
                    yield
                for mo in range(16):
                    fc1_mm_unit(0, mo, staged=True)
                    yield
                gelu_island(0, 0, 16)      # ~11us Act island
                for mo in range(16, n_mo):
                    fc1_mm_unit(0, mo, staged=True)
                    yield
                gelu_island(0, 16, n_mo)   # second island
                yield
                yield
                yield from fc2_gen(0)

            g0 = mlp0_gen()
            pull0 = make_pull(g0)
            for hp in range(n_hp):
                attn_block(1, hp, pull=pull0, fsc=1,
                           fctx=(1 if hp >= 5 else 0))
            for _ in g0:
                pass

            # ---------------- tail: qq=1 MLP ------------------------------
            w1_map.clear()
            w1_need(0)
            for mo in range(n_dc):
                wo_unit(1, mo)
            ln2_block(1)
            for mo in range(n_mo):
                fc1_plain_unit(1, mo)
            for _ in fc2_gen(1):
                pass
    nc.compile()
    return nc


_NC_CACHE = {}


def _get_nc(T, Q, Dm, Hh, Mlp, n_cores,
            scales=(16.0, 16.0, 16.0, 16.0, 16.0, 16.0, 3.5)):
    key = (T, Q, Dm, Hh, Mlp, n_cores, tuple(scales))
    if key not in _NC_CACHE:
        _NC_CACHE[key] = build_bass(T, Q, Dm, Hh, Mlp, n_cores, scales)
    return _NC_CACHE[key]


def _split_f8(w):
    hi = w.astype(NP_F8)
    lo = (w - hi.astype(np.float32)).astype(NP_F8)
    return np.stack([hi, lo])


def _pow2_scale(absmax, target=128.0):
    a = float(absmax)
    if not np.isfinite(a) or a <= 0:
        return 1.0
    return float(2.0 ** math.floor(math.log2(target / a)))


def _swz_qk(w8):
    # (D, D) -> [p][mo][cj][j][m] SBUF image, flattened to (P, 8192)
    t = w8.reshape(4, 2, P, 8, P)           # (c, j, p, mo, m)
    return np.ascontiguousarray(t.transpose(2, 3, 0, 1, 4)).reshape(P, -1)


def _swz_v(w8):
    t = w8.reshape(4, 2, P, 2, 512)         # (c, j, p, no, m)
    return np.ascontiguousarray(t.transpose(2, 3, 0, 1, 4)).reshape(P, -1)


def _swz_wo(w8):
    t = w8.reshape(4, 2, P, 2, 4, P)        # (c, j, p, g, mo, m)
    return np.ascontiguousarray(t.transpose(3, 2, 0, 1, 4, 5)).reshape(
        2, P, -1)


def _swz_w1(w8):
    t = w8.reshape(4, 2, P, 8, 4, P)        # (c, j, p, g, mo, m)
    return np.ascontiguousarray(t.transpose(3, 2, 0, 1, 4, 5)).reshape(
        8, P, -1)


def _swz_w2(w8_2):
    # (2, MLP, D) -> (2, 8, P, 4096): [s][mo2][p][c][j][m]
    t = w8_2.reshape(2, 16, 2, P, 8, P)     # (s, c, j, p, mo2, m)
    return np.ascontiguousarray(t.transpose(0, 4, 3, 1, 2, 5)).reshape(
        2, 8, P, -1)


def prepare(inputs):
    """Host-side prep: LN folding, fp8 quantization, per-core input maps."""
    f = lambda k: np.asarray(inputs[k], np.float32)
    x = f("x")
    Bq, Sq, Dq = x.shape
    Qtok = Sq // 2
    g1, b1ln = f("ln1_g"), f("ln1_b")
    g2, b2ln = f("ln2_g"), f("ln2_b")
    Wq, Wk, Wv, Wo = f("Wq"), f("Wk"), f("Wv"), f("Wo")
    W1, W2 = f("W1"), f("W2")
    bq, bk, bv, bo = f("bq"), f("bk"), f("bv"), f("bo")
    b1, b2 = f("b1"), f("b2")

    # fold LN1 gain/bias into QKV, LN2 gain/bias into W1 (exact)
    Wq_e = g1[:, None] * Wq
    Wk_e = g1[:, None] * Wk
    Wv_e = g1[:, None] * Wv
    bq_e = bq + b1ln @ Wq
    bk_e = bk + b1ln @ Wk
    bv_e = bv + b1ln @ Wv
    W1_e = g2[:, None] * W1
    b1_e = b1 + b2ln @ W1

    s_wq = _pow2_scale(np.abs(Wq_e).max())
    s_wk = _pow2_scale(np.abs(Wk_e).max())
    # V result is stored in fp8 still scaled by s_wv: bound both weight and
    # activation range (sigma of v_j ~ col norm of Wv_e, x is LN'd)
    vcol = np.sqrt((Wv_e ** 2).sum(0))
    vmag = max(float((vcol * 8).max()), float(np.abs(bv_e).max() * 4), 1e-6)
    s_wv = min(_pow2_scale(np.abs(Wv_e).max()),
               _pow2_scale(vmag, target=200.0))
    s_wo = _pow2_scale(np.abs(Wo).max())
    s_w1 = _pow2_scale(np.abs(W1_e).max())
    s_w2 = _pow2_scale(np.abs(W2).max())

    # estimate max attention score for the exp shift C (sampled)
    mu = x.mean(-1, keepdims=True)
    va = x.var(-1, keepdims=True)
    xn_h = (x - mu) / np.sqrt(va + EPS)
    qi = xn_h[:, ::89][:, :16].reshape(-1, Dq)
    ki = xn_h[:, ::13][:, :128].reshape(-1, Dq)
    qp = (qi @ Wq_e + bq_e).reshape(Bq, -1, H, Dq // H)
    kp = (ki @ Wk_e + bk_e).reshape(Bq, -1, H, Dq // H)
    sc = np.einsum("bqhd,bkhd->bhqk", qp, kp) / np.sqrt(Dq // H)
    shift_c = float(sc.max() + 2.0 * sc.std() - math.log(200.0))

    scales = (s_wq, s_wk, s_wv, s_wo, s_w1, s_w2, shift_c)
    nc = _get_nc(Sq, Qtok, Dq, H, MLP, N_CORES, scales)

    shared = {
        "wq8": _swz_qk((Wq_e * s_wq).astype(NP_F8)),
        "wk8": _swz_qk((Wk_e * s_wk).astype(NP_F8)),
        "wv8": _swz_v((Wv_e * s_wv).astype(NP_F8)),
        "wo8": _swz_wo((Wo * s_wo).astype(NP_F8)),
        "w18": _swz_w1((W1_e * s_w1).astype(NP_F8)),
        "w28": _swz_w2(_split_f8(W2 * s_w2)),
        "bq": (bq_e * s_wq).astype(np.float32),
        "bk": (bk_e * s_wk).astype(np.float32),
        "bv16": (bv_e * s_wv).astype(ml_dtypes.bfloat16),
        "bo": bo.astype(np.float32),
        "b1": b1_e.astype(np.float32),
        "b2": b2.astype(np.float32),
        "ones32": np.ones((P, P), np.float32),
    }
    in_maps = []
    for c in range(N_CORES):
        b = c // 2
        half = c % 2
        xb = x[b]
        xr = np.concatenate(
            [xb[half * Qtok:(half + 1) * Qtok],
             xb[(1 - half) * Qtok:(2 - half) * Qtok]], axis=0)
        m = dict(shared)
        m["xT"] = np.ascontiguousarray(xr.T)
        in_maps.append(m)
    return nc, in_maps, Qtok


def unshard(res, Bq, Sq, Dq, Qtok):
    out = np.empty((Bq, Sq, Dq), np.float32)
    for c in range(N_CORES):
        b = c // 2
        half = c % 2
        out[b, half * Qtok:(half + 1) * Qtok, :] = res.results[c]["yT"].T
    return out


def kernel(**inputs):
    x = np.asarray(inputs["x"], np.float32)
    Bq, Sq, Dq = x.shape
    nc, in_maps, Qtok = prepare(inputs)
    res = run_bass_kernel_spmd(nc, in_maps, core_ids=list(range(N_CORES)))
    return unshard(res, Bq, Sq, Dq, Qtok)


# revision 29
# speedup vs baseline: 1.0357x; 1.0357x over previous
"""Trainium2 Bass kernel for a dense transformer block (LN1 -> MHA -> LN2 -> MLP).

Sharding: 8 cores = (batch b in 0..3) x (sequence half in 0..1), zero
cross-core communication. Each core's input tokens are reordered on the host
so its 1024 query tokens are always tokens 0..1023 of its 2048-token view
(key/value order is irrelevant to attention), letting one SPMD program serve
every core and the query-side LN reuse the full-sequence LN output.

Precision: fp8e4m3 DoubleRow matmuls for QKV/O projections, ctx, and the MLP
(weights pre-scaled by power-of-2 factors on the host; descales fold into
existing bias/scale stages, so they cost nothing). Scores stay bf16.
LayerNorm gain/bias are folded into the following weights on the host
(mathematically exact), so the device LN is a pure (x-mu)*rstd normalize.

Softmax: exp(score - C) with a host-estimated shift C keeping exp outputs in
fp8 range; the denominator is produced by a ones-column appended to V inside
the ctx DoubleRow matmul (out partition 65), so it costs no extra PE time.

Schedule: the Act engine's exp stream (16 blocks x ~16us) is the backbone;
everything else is emitted as "filler units" pulled into slots between the
per-kcp score/ctx groups of each attention block so the in-order PE queue
stays fed without ever delaying the next scores (which would starve Act).
Projections stream into the qq=0 attention blocks; wo/ln2/fc1 of qq=0 stream
into the qq=1 blocks. fc1 results are staged pre-gelu in bf16 (aliasing the
Y1 arena via bitcast) so gelus run as two contiguous islands (2 activation-
table switches instead of ~64); fc2(0) fills the late qq=1 blocks. Weights
are pre-swizzled on the host into the exact SBUF tile images so every weight
DMA is fully contiguous (2x descriptor throughput vs 128B strides).
"""

import math
import sys

if '/opt/trn_rl_repo' not in sys.path:
    sys.path.insert(0, '/opt/trn_rl_repo')

import numpy as np
import ml_dtypes

import concourse.tile as tile
import concourse.mybir as mybir
from concourse import bacc
from concourse.bass import ts
from concourse.bass_utils import run_bass_kernel_spmd

P = 128
F32 = mybir.dt.float32
F32R = mybir.dt.float32r
BF16 = mybir.dt.bfloat16
F8 = mybir.dt.float8e4
AF = mybir.ActivationFunctionType
DR = mybir.MatmulPerfMode.DoubleRow
ALU = mybir.AluOpType
EPS = 1e-6

B, S, D, H, MLP = 4, 2048, 1024, 16, 4096
N_CORES = 8
NP_F8 = ml_dtypes.float8_e4m3


def build_bass(T, Q, Dm, Hh, Mlp, n_cores, scales, dbg=False):
    s_wq, s_wk, s_wv, s_wo, s_w1, s_w2, shift_c = scales
    dh = Dm // Hh
    assert dh == 64
    n_dc = Dm // P          # 8 feature chunks
    n_cj = n_dc // 2        # 4 DoubleRow k-pair steps over D
    n_tk = T // P           # 16 token chunks
    TB = 512
    n_tb = T // TB          # 4
    KTB = 512               # K projection token slice
    n_ktb = T // KTB        # 4
    QQ = 512
    n_qq = Q // QQ          # 2
    n_mo = Mlp // P         # 32
    n_m2 = n_mo // 2        # 16 DoubleRow k-pair steps over MLP
    n_hp = Hh // 2          # 8 head pairs
    NO = 512
    inv_d = 1.0 / Dm
    exp_scale = 0.125 / (s_wq * s_wk)
    c_wo = 1.0 / (s_wo * s_wv)
    inv_s1 = 1.0 / s_w1
    inv_s2 = 1.0 / s_w2

    nc = bacc.Bacc("TRN2", target_bir_lowering=False, debug=False,
                   enable_asserts=False, num_devices=n_cores)

    def din(name, shape, dt):
        return nc.dram_tensor(name, shape, dt, kind="ExternalInput").ap()

    xT_d = din("xT", (Dm, T), F32)
    # host-swizzled weight images: per-partition-contiguous SBUF tile layouts
    wq_d = din("wq8", (P, n_dc * n_cj * 2 * P), F8)
    wk_d = din("wk8", (P, n_dc * n_cj * 2 * P), F8)
    wv_d = din("wv8", (P, 2 * n_cj * 2 * NO), F8)
    wo_d = din("wo8", (2, P, n_cj * 2 * 4 * P), F8)
    w1_d = din("w18", (n_mo // 4, P, n_cj * 2 * 4 * P), F8)
    w2_d = din("w28", (2, n_dc, P, n_m2 * 2 * P), F8)
    bq_d, bk_d = din("bq", (Dm,), F32), din("bk", (Dm,), F32)
    bv_d, bo_d = din("bv16", (Dm,), BF16), din("bo", (Dm,), F32)
    b1_d, b2_d = din("b1", (Mlp,), F32), din("b2", (Dm,), F32)
    ones_d = din("ones32", (P, P), F32)
    yT_d = nc.dram_tensor("yT", (Dm, Q), F32, kind="ExternalOutput").ap()

    with tile.TileContext(nc) as tc, \
         tc.tile_pool(name="const", bufs=1) as constp, \
         tc.tile_pool(name="p_res", bufs=1) as p_res, \
         tc.tile_pool(name="p_kv", bufs=1) as p_kv, \
         tc.tile_pool(name="p_exp", bufs=2) as p_exp, \
         tc.tile_pool(name="p_rb", bufs=1) as p_rb, \
         tc.tile_pool(name="p_ct", bufs=1) as p_ct, \
         tc.tile_pool(name="p_wos", bufs=2) as p_wos, \
         tc.tile_pool(name="p_w1", bufs=2) as p_w1, \
         tc.tile_pool(name="p_out", bufs=1) as p_out, \
         tc.tile_pool(name="ps_sc", bufs=2, space="PSUM") as ps_sc, \
         tc.tile_pool(name="ps_ctx", bufs=2, space="PSUM") as ps_ctx, \
         tc.tile_pool(name="ps_sh", bufs=2, space="PSUM") as ps_sh:

        ones_f = constp.tile([P, P], BF16)
        nc.vector.memset(ones_f[:], 1.0)
        negc_t = constp.tile([P, 1], F32)
        nc.vector.memset(negc_t[:], -shift_c)

        def vec_tile(src, n, nm):
            t = constp.tile([P, n], F32, tag=nm, name=nm)
            nc.sync.dma_start(t[:], src.rearrange("(c p) -> p c", p=P))
            return t

        bq_t, bk_t = vec_tile(bq_d, n_dc, "bq"), vec_tile(bk_d, n_dc, "bk")
        bo_t, b2_t = vec_tile(bo_d, n_dc, "bo"), vec_tile(b2_d, n_dc, "b2")
        b1_t = vec_tile(b1_d, n_mo, "b1")

        XQ = p_res.tile([P, n_dc, Q], BF16)       # residual stream (bf16)
        KT = p_kv.tile([P, n_dc, T], BF16)
        QT = p_kv.tile([P, n_dc, Q], BF16)
        VT = p_kv.tile([P, n_tk, Hh, 65], F8)
        nc.gpsimd.memset(VT[:, :, :, 64:65], 1.0)
        CT8 = p_ct.tile([P, n_dc, Q], F8)

        def attn_block(qq, hp, pull=None, fsc=1, fctx=1):
            """One head-pair of attention for query chunk qq, software-
            pipelined, with filler slots after each score/ctx group."""
            qsl = ts(qq, QQ)
            exps = [p_exp.tile([P, n_tk, QQ], F8, tag="expT", name="expT")
                    for _ in range(2)]
            pcs = [ps_ctx.tile([65, QQ], F32, tag="ps_c", name="ps_c")
                   for _ in range(2)]
            nk2 = n_tk // 2
            LAG = 2
            for j in range(nk2 + LAG):
                if j < nk2:
                    for hi in range(2):
                        r0 = hi * 64
                        pss = ps_sc.tile([P, 2, QQ], F32, tag="ps_s",
                                         name="ps_s")
                        for jj in range(2):
                            nc.tensor.matmul(
                                pss[:, jj, :],
                                KT[r0:r0 + 64, hp, ts(2 * j + jj, P)],
                                QT[r0:r0 + 64, hp, qsl],
                                start=True, stop=True)
                        nc.scalar.activation(
                            exps[hi][:, 2 * j:2 * j + 2, :],
                            pss[:, :, :], AF.Exp,
                            scale=exp_scale, bias=negc_t[:, 0:1])
                    if pull:
                        pull(fsc)
                if j >= LAG:
                    kcp = j - LAG
                    for hi in range(2):
                        h = 2 * hp + hi
                        nc.tensor.matmul(
                            pcs[hi][:, :],
                            VT[:, 2 * kcp:2 * kcp + 2, h, 0:65],
                            exps[hi][:, 2 * kcp:2 * kcp + 2, :],
                            start=(kcp == 0), stop=(kcp == nk2 - 1),
                            perf_mode=DR)
                    if pull:
                        pull(fctx)
            for hi in range(2):
                r0 = hi * 64
                rb1 = p_rb.tile([1, QQ], F32, tag="rb1")
                nc.vector.reciprocal(rb1[:], pcs[hi][64:65, :])
                rbb = p_rb.tile([64, QQ], F32, tag="rbb")
                nc.gpsimd.partition_broadcast(rbb[:], rb1[:])
                nc.vector.tensor_mul(CT8[r0:r0 + 64, hp, qsl],
                                     pcs[hi][0:64, :], rbb[:])

        def make_pull(gen):
            done = [False]
            def pull(n=1):
                if done[0]:
                    return
                for _ in range(n):
                    try:
                        next(gen)
                    except StopIteration:
                        done[0] = True
                        return
            return pull

        w1_map = {}

        def w1_need(g):
            if g not in w1_map:
                wt = p_w1.tile([P, n_cj, 2, 4, P], F8, tag="w1")
                nc.sync.dma_start(
                    wt[:].rearrange("p a b c d -> p (a b c d)"), w1_d[g])
                w1_map[g] = wt
            return w1_map[g]

        # ================= Phase A: LN1 + projections + attn(qq=0) =========
        with tc.tile_pool(name="p_xn8", bufs=1) as p_xn8, \
             tc.tile_pool(name="p_x", bufs=2) as p_x, \
             tc.tile_pool(name="p_t", bufs=2) as p_t, \
             tc.tile_pool(name="p_wk", bufs=1) as p_wk:

            XN8 = p_xn8.tile([P, n_dc, T], F8)
            bv_row = p_xn8.tile([1, Dm], BF16)
            nc.sync.dma_start(bv_row[:, :], bv_d[None, :])
            bv_bc = p_xn8.tile([P, Dm], BF16)
            nc.gpsimd.partition_broadcast(bv_bc[:], bv_row[:])

            wk_t = p_wk.tile([P, n_dc, n_cj, 2, P], F8, tag="wk")
            nc.sync.dma_start(
                wk_t[:].rearrange("p a b c d -> p (a b c d)"), wk_d[:, :])
            wq_t = p_wk.tile([P, n_dc, n_cj, 2, P], F8, tag="wq")
            nc.sync.dma_start(
                wq_t[:].rearrange("p a b c d -> p (a b c d)"), wq_d[:, :])
            wv_t = p_wk.tile([P, 2, n_cj, 2, NO], F8, tag="wv")
            nc.sync.dma_start(
                wv_t[:].rearrange("p a b c d -> p (a b c d)"), wv_d[:, :])

            def k_unit(mo, tb):
                tsl = ts(tb, KTB)
                ps = ps_sh.tile([P, KTB], F32, tag="sh")
                for cj in range(n_cj):
                    nc.tensor.matmul(
                        ps[:], wk_t[:, mo, cj, :, :],
                        XN8[:, 2 * cj:2 * cj + 2, tsl],
                        start=(cj == 0), stop=(cj == n_cj - 1),
                        perf_mode=DR)
                nc.vector.tensor_scalar_add(KT[:, mo, tsl], ps[:],
                                            bk_t[:, mo:mo + 1])

            def q_unit(mo, qb):
                qsl = ts(qb, QQ)
                ps = ps_sh.tile([P, QQ], F32, tag="sh")
                for cj in range(n_cj):
                    nc.tensor.matmul(
                        ps[:], wq_t[:, mo, cj, :, :],
                        XN8[:, 2 * cj:2 * cj + 2, qsl],
                        start=(cj == 0), stop=(cj == n_cj - 1),
                        perf_mode=DR)
                nc.vector.tensor_scalar_add(QT[:, mo, qsl], ps[:],
                                            bq_t[:, mo:mo + 1])

            def v_unit(no, to):
                ps = ps_sh.tile([P, NO], F32, tag="sh")
                for cj in range(n_cj):
                    nc.tensor.matmul(
                        ps[:], XN8[:, 2 * cj:2 * cj + 2, ts(to, P)],
                        wv_t[:, no, cj, :, :],
                        start=(cj == 0), stop=(cj == n_cj - 1),
                        perf_mode=DR)
                nc.vector.tensor_add(VT[:, to, 8 * no:8 * no + 8, 0:64],
                                     ps[:], bv_bc[:, ts(no, NO)])

            # ---- LN1 per token block; first projections ride along ----
            for tb in range(n_tb):
                tsl = ts(tb, TB)
                xc = p_x.tile([P, n_dc, TB], BF16, tag="xc")
                for dc in range(n_dc):
                    nc.gpsimd.dma_start(xc[:, dc, :], xT_d[ts(dc, P), tsl])
                st = ps_sc.tile([P, 2, TB], F32, tag="ps_s", name="ps_s")
                for dc in range(n_dc):
                    nc.tensor.matmul(st[:, 0, :], ones_f[:], xc[:, dc, :],
                                     start=(dc == 0), stop=(dc == n_dc - 1))
                    xsq = p_t.tile([P, TB], BF16, tag="xsq")
                    nc.scalar.activation(xsq[:], xc[:, dc, :], AF.Square)
                    nc.tensor.matmul(st[:, 1, :], ones_f[:], xsq[:],
                                     start=(dc == 0), stop=(dc == n_dc - 1))
                    if tb < Q // TB:
                        nc.vector.tensor_copy(XQ[:, dc, tsl], xc[:, dc, :])
                mbc = p_t.tile([P, TB], F32, tag="mbc")
                nc.vector.tensor_scalar_mul(mbc[:], st[:, 0, :], inv_d)
                var = p_t.tile([P, TB], F32, tag="var")
                nc.vector.tensor_scalar(var[:], st[:, 1, :], inv_d, EPS,
                                        op0=ALU.mult, op1=ALU.add)
                m2 = p_t.tile([P, TB], F32, tag="tn")
                nc.vector.tensor_mul(m2[:], mbc[:], mbc[:])
                nc.vector.tensor_sub(var[:], var[:], m2[:])
                nc.scalar.activation(var[:], var[:], AF.Sqrt)
                rstd = p_t.tile([P, TB], F32, tag="rstd")
                nc.vector.reciprocal(rstd[:], var[:])
                for dc in range(n_dc):
                    t0 = p_t.tile([P, TB], F32, tag="tn")
                    nc.vector.tensor_sub(t0[:], xc[:, dc, :], mbc[:])
                    nc.gpsimd.tensor_mul(XN8[:, dc, tsl], t0[:], rstd[:])
                # K head-pair 0 + first V chunks ride each LN1 block
                for to in range(4 * tb, 4 * tb + 4):
                    v_unit(0, to)
                k_unit(0, tb)
            q_unit(0, 0)

            def projection_gen():
                # one ~0.4-0.9us unit per yield; ordered so k(hp)/q(hp,0)
                # land before attn(0,hp) and v(no1) before ctx of attn(0,4)
                for mo in range(1, 5):
                    for tb in range(n_ktb):
                        k_unit(mo, tb)
                        yield
                    q_unit(mo, 0)
                    yield
                for to in range(8):
                    v_unit(1, to)
                    yield
                for tb in range(n_ktb):
                    k_unit(5, tb)
                    yield
                q_unit(5, 0)
                yield
                for to in range(8, 16):
                    v_unit(1, to)
                    yield
                for mo in (6, 7):
                    for tb in range(n_ktb):
                        k_unit(mo, tb)
                        yield
                    q_unit(mo, 0)
                    yield
                for mo in range(n_dc):
                    q_unit(mo, 1)
                    yield
                # prefetch wo groups + w1 group 0 for phase B
                for g in range(2):
                    wt = p_wos.tile([P, n_cj, 2, 4, P], F8, tag="wo")
                    nc.sync.dma_start(
                        wt[:].rearrange("p a b c d -> p (a b c d)"),
                        wo_d[g])
                    wo_tiles.append(wt)
                yield
                w1_need(0)
                yield

            wo_tiles = []
            gA = projection_gen()
            pullA = make_pull(gA)
            for hp in range(n_hp):
                attn_block(0, hp, pull=pullA, fsc=1, fctx=0)
            for _ in gA:
                pass

        # ================= Phase B: attn(qq=1) + MLP =======================
        with tc.tile_pool(name="p_mlp", bufs=1) as p_mlp, \
             tc.tile_pool(name="p_t2", bufs=2) as p_t2, \
             tc.tile_pool(name="p_t2s", bufs=1) as p_t2s, \
             tc.tile_pool(name="p_w2", bufs=2) as p_w2, \
             tc.tile_pool(name="p_y1", bufs=1) as p_y1:

            XN2 = p_mlp.tile([P, n_dc, 2, Q], F8)  # [.., hi/lo, ..]
            y1s = {}

            def wo_unit(qq, mo):
                qsl = ts(qq, QQ)
                ps = ps_sh.tile([P, QQ], F32, tag="sh")
                for cj in range(n_cj):
                    nc.tensor.matmul(
                        ps[:], wo_tiles[mo // 4][:, cj, :, mo % 4, :],
                        CT8[:, 2 * cj:2 * cj + 2, qsl],
                        start=(cj == 0), stop=(cj == n_cj - 1),
                        perf_mode=DR)
                tw = p_out.tile([P, QQ], F32, tag="ot")
                nc.vector.tensor_scalar(tw[:], ps[:], c_wo,
                                        bo_t[:, mo:mo + 1],
                                        op0=ALU.mult, op1=ALU.add)
                nc.vector.tensor_add(XQ[:, mo, qsl], tw[:], XQ[:, mo, qsl])

            def ln2_block(qq):
                qsl = ts(qq, QQ)
                st2 = ps_sc.tile([P, 2, QQ], F32, tag="ps_s", name="ps_s")
                for dc in range(n_dc):
                    nc.tensor.matmul(st2[:, 0, :], ones_f[:], XQ[:, dc, qsl],
                                     start=(dc == 0), stop=(dc == n_dc - 1))
                    sq = p_t2.tile([P, QQ], BF16, tag="sq2")
                    nc.gpsimd.tensor_mul(sq[:], XQ[:, dc, qsl],
                                         XQ[:, dc, qsl])
                    nc.tensor.matmul(st2[:, 1, :], ones_f[:], sq[:],
                                     start=(dc == 0), stop=(dc == n_dc - 1))
                mbc = p_t2s.tile([P, QQ], F32, tag="mbc2")
                nc.vector.tensor_scalar_mul(mbc[:], st2[:, 0, :], inv_d)
                var = p_t2s.tile([P, QQ], F32, tag="var2")
                nc.vector.tensor_scalar(var[:], st2[:, 1, :], inv_d, EPS,
                                        op0=ALU.mult, op1=ALU.add)
                m2 = p_t2.tile([P, QQ], F32, tag="tn2")
                nc.vector.tensor_mul(m2[:], mbc[:], mbc[:])
                nc.vector.tensor_sub(var[:], var[:], m2[:])
                # rstd = rsqrt(var) on DVE only: seed from 1/var + Newton
                r = p_t2s.tile([P, QQ], F32, tag="rstd2")
                nc.vector.reciprocal_approx_fast(r[:], var[:])
                nc.vector.tensor_scalar(r[:], r[:], 0.72, 0.35,
                                        op0=ALU.mult, op1=ALU.add)
                for _ in range(3):
                    t1 = p_t2.tile([P, QQ], F32, tag="tn2")
                    nc.vector.tensor_mul(t1[:], r[:], r[:])
                    nc.vector.tensor_mul(t1[:], t1[:], var[:])
                    nc.vector.tensor_scalar(t1[:], t1[:], -0.5, 1.5,
                                            op0=ALU.mult, op1=ALU.add)
                    nc.vector.tensor_mul(r[:], r[:], t1[:])
                for dc in range(n_dc):
                    t0 = p_t2.tile([P, QQ], F32, tag="tn2")
                    nc.gpsimd.tensor_sub(t0[:], XQ[:, dc, qsl], mbc[:])
                    m = p_t2.tile([P, QQ], F32, tag="m32")
                    nc.vector.tensor_mul(m[:], t0[:], r[:])
                    nc.vector.tensor_copy(XN2[:, dc, 0, qsl], m[:])
                    nc.gpsimd.tensor_sub(XN2[:, dc, 1, qsl], m[:],
                                         XN2[:, dc, 0, qsl])

            def y1_tile(qq):
                if qq not in y1s:
                    y1s[qq] = p_y1.tile([P, n_mo, 2, QQ], F8, tag="y1",
                                        name="y1")
                return y1s[qq]

            def z1_view(Y1, mo):
                # bf16 view of Y1[:, mo, :, :]'s bytes (pre-gelu staging)
                return Y1[:, mo, :, :].bitcast(BF16).rearrange(
                    "p a b -> p (a b)")

            def fc1_mm_unit(qq, mo, staged):
                qsl = ts(qq, QQ)
                Y1 = y1_tile(qq)
                wt = w1_need(mo // 4)
                if mo % 4 == 0 and (mo // 4) + 1 < n_mo // 4:
                    w1_need((mo // 4) + 1)   # prefetch next group
                ps = ps_sh.tile([P, QQ], F32, tag="sh")
                for lv in range(2):
                    for cj in range(n_cj):
                        nc.tensor.matmul(
                            ps[:], wt[:, cj, :, mo % 4, :],
                            XN2[:, 2 * cj:2 * cj + 2, lv, qsl],
                            start=(lv == 0 and cj == 0),
                            stop=(lv == 1 and cj == n_cj - 1),
                            perf_mode=DR)
                if staged:
                    nc.vector.tensor_copy(z1_view(Y1, mo), ps[:])
                    return None
                return ps

            def gelu_island(qq, mo0, mo1):
                Y1 = y1_tile(qq)
                for mo in range(mo0, mo1):
                    g32 = p_t2.tile([P, QQ], F32, tag="m32")
                    nc.scalar.activation(g32[:], z1_view(Y1, mo), AF.Gelu,
                                         bias=b1_t[:, mo:mo + 1],
                                         scale=inv_s1)
                    nc.vector.tensor_copy(Y1[:, mo, 0, :], g32[:])
                    nc.gpsimd.tensor_sub(Y1[:, mo, 1, :], g32[:],
                                         Y1[:, mo, 0, :])
                # fence: rewrite negc_t (same value) from the island's last
                # split so every later exp waits for the whole island --
                # keeps the gelu run contiguous (one act-table switch each
                # way) under the readiness-greedy tile scheduler
                nc.vector.tensor_scalar(negc_t[:, 0:1],
                                        Y1[:, mo1 - 1, 1, 0:1], 0.0,
                                        -shift_c, op0=ALU.mult, op1=ALU.add)

            def fc1_plain_unit(qq, mo):
                # tail variant: inline gelu (gelus end up consecutive)
                Y1 = y1_tile(qq)
                ps = fc1_mm_unit(qq, mo, staged=False)
                g32 = p_t2.tile([P, QQ], F32, tag="m32")
                nc.scalar.activation(g32[:], ps[:], AF.Gelu,
                                     bias=b1_t[:, mo:mo + 1],
                                     scale=inv_s1)
                nc.vector.tensor_copy(Y1[:, mo, 0, :], g32[:])
                nc.gpsimd.tensor_sub(Y1[:, mo, 1, :], g32[:],
                                     Y1[:, mo, 0, :])

            def fc2_gen(qq, m0=0, m1=None):
                qsl = ts(qq, QQ)
                if m1 is None:
                    m1 = n_dc
                Y1 = y1s[qq]
                if m1 == n_dc:
                    y1s.pop(qq)
                for mo2 in range(m0, m1):
                    wth = p_w2.tile([P, n_m2, 2, P], F8, tag="w2h")
                    nc.sync.dma_start(
                        wth[:].rearrange("p a b c -> p (a b c)"),
                        w2_d[0, mo2])
                    wtl = p_w2.tile([P, n_m2, 2, P], F8, tag="w2l")
                    nc.sync.dma_start(
                        wtl[:].rearrange("p a b c -> p (a b c)"),
                        w2_d[1, mo2])
                    ps = ps_sh.tile([P, QQ], F32, tag="sh")
                    terms = [(wth, 0), (wth, 1), (wtl, 0)]
                    for ti, (wt, lv) in enumerate(terms):
                        for cj in range(n_m2):
                            nc.tensor.matmul(
                                ps[:], wt[:, cj, :, :],
                                Y1[:, 2 * cj:2 * cj + 2, lv, :],
                                start=(ti == 0 and cj == 0),
                                stop=(ti == 2 and cj == n_m2 - 1),
                                perf_mode=DR)
                        yield
                    ot = p_out.tile([P, QQ], F32, tag="ot")
                    nc.vector.tensor_scalar(ot[:], ps[:], inv_s2,
                                            b2_t[:, mo2:mo2 + 1],
                                            op0=ALU.mult, op1=ALU.add)
                    nc.vector.tensor_add(ot[:], ot[:], XQ[:, mo2, qsl])
                    nc.sync.dma_start(yT_d[ts(mo2, P), qsl], ot[:])

            def mlp0_gen():
                # stretch fillers for qq=0 MLP, pulled into attn(1,*) slots
                for mo in range(n_dc):
                    wo_unit(0, mo)
                    yield
                ln2_block(0)
                # bubbles: let the DVE/Pool XN2 chain finish before the
                # first fc1 matmul hits the in-order PE queue
                for _ in range(12):
                    yield
                for mo in range(16):
                    fc1_mm_unit(0, mo, staged=True)
                    yield
                gelu_island(0, 0, 16)      # ~11us Act island
                for mo in range(16, n_mo):
                    fc1_mm_unit(0, mo, staged=True)
                    yield
                gelu_island(0, 16, n_mo)   # second island
                yield
                yield
                yield from fc2_gen(0, 0, 6)

            g0 = mlp0_gen()
            pull0 = make_pull(g0)
            for hp in range(n_hp):
                attn_block(1, hp, pull=pull0, fsc=1,
                           fctx=(1 if hp >= 5 else 0))
            for _ in g0:
                pass

            # ---------------- tail: qq=1 MLP ------------------------------
            w1_map.clear()
            for mo in range(n_dc):
                wo_unit(1, mo)
            ln2_block(1)
            for _ in fc2_gen(0, 6, 8):   # overlaps the XN2(1) latency chain
                pass
            w1_need(0)
            for mo in range(n_mo):
                fc1_plain_unit(1, mo)
            for _ in fc2_gen(1):
                pass
    nc.compile()
    return nc


_NC_CACHE = {}


def _get_nc(T, Q, Dm, Hh, Mlp, n_cores,
            scales=(16.0, 16.0, 16.0, 16.0, 16.0, 16.0, 3.5)):
    key = (T, Q, Dm, Hh, Mlp, n_cores, tuple(scales))
    if key not in _NC_CACHE:
        _NC_CACHE[key] = build_bass(T, Q, Dm, Hh, Mlp, n_cores, scales)
    return _NC_CACHE[key]


def _split_f8(w):
    hi = w.astype(NP_F8)
    lo = (w - hi.astype(np.float32)).astype(NP_F8)
    return np.stack([hi, lo])


def _pow2_scale(absmax, target=128.0):
    a = float(absmax)
    if not np.isfinite(a) or a <= 0:
        return 1.0
    return float(2.0 ** math.floor(math.log2(target / a)))


def _swz_qk(w8):
    # (D, D) -> [p][mo][cj][j][m] SBUF image, flattened to (P, 8192)
    t = w8.reshape(4, 2, P, 8, P)           # (c, j, p, mo, m)
    return np.ascontiguousarray(t.transpose(2, 3, 0, 1, 4)).reshape(P, -1)


def _swz_v(w8):
    t = w8.reshape(4, 2, P, 2, 512)         # (c, j, p, no, m)
    return np.ascontiguousarray(t.transpose(2, 3, 0, 1, 4)).reshape(P, -1)


def _swz_wo(w8):
    t = w8.reshape(4, 2, P, 2, 4, P)        # (c, j, p, g, mo, m)
    return np.ascontiguousarray(t.transpose(3, 2, 0, 1, 4, 5)).reshape(
        2, P, -1)


def _swz_w1(w8):
    t = w8.reshape(4, 2, P, 8, 4, P)        # (c, j, p, g, mo, m)
    return np.ascontiguousarray(t.transpose(3, 2, 0, 1, 4, 5)).reshape(
        8, P, -1)


def _swz_w2(w8_2):
    # (2, MLP, D) -> (2, 8, P, 4096): [s][mo2][p][c][j][m]
    t = w8_2.reshape(2, 16, 2, P, 8, P)     # (s, c, j, p, mo2, m)
    return np.ascontiguousarray(t.transpose(0, 4, 3, 1, 2, 5)).reshape(
        2, 8, P, -1)


def prepare(inputs):
    """Host-side prep: LN folding, fp8 quantization, per-core input maps."""
    f = lambda k: np.asarray(inputs[k], np.float32)
    x = f("x")
    Bq, Sq, Dq = x.shape
    Qtok = Sq // 2
    g1, b1ln = f("ln1_g"), f("ln1_b")
    g2, b2ln = f("ln2_g"), f("ln2_b")
    Wq, Wk, Wv, Wo = f("Wq"), f("Wk"), f("Wv"), f("Wo")
    W1, W2 = f("W1"), f("W2")
    bq, bk, bv, bo = f("bq"), f("bk"), f("bv"), f("bo")
    b1, b2 = f("b1"), f("b2")

    # fold LN1 gain/bias into QKV, LN2 gain/bias into W1 (exact)
    Wq_e = g1[:, None] * Wq
    Wk_e = g1[:, None] * Wk
    Wv_e = g1[:, None] * Wv
    bq_e = bq + b1ln @ Wq
    bk_e = bk + b1ln @ Wk
    bv_e = bv + b1ln @ Wv
    W1_e = g2[:, None] * W1
    b1_e = b1 + b2ln @ W1

    s_wq = _pow2_scale(np.abs(Wq_e).max())
    s_wk = _pow2_scale(np.abs(Wk_e).max())
    # V result is stored in fp8 still scaled by s_wv: bound both weight and
    # activation range (sigma of v_j ~ col norm of Wv_e, x is LN'd)
    vcol = np.sqrt((Wv_e ** 2).sum(0))
    vmag = max(float((vcol * 8).max()), float(np.abs(bv_e).max() * 4), 1e-6)
    s_wv = min(_pow2_scale(np.abs(Wv_e).max()),
               _pow2_scale(vmag, target=200.0))
    s_wo = _pow2_scale(np.abs(Wo).max())
    s_w1 = _pow2_scale(np.abs(W1_e).max())
    s_w2 = _pow2_scale(np.abs(W2).max())

    # estimate max attention score for the exp shift C (sampled)
    mu = x.mean(-1, keepdims=True)
    va = x.var(-1, keepdims=True)
    xn_h = (x - mu) / np.sqrt(va + EPS)
    qi = xn_h[:, ::89][:, :16].reshape(-1, Dq)
    ki = xn_h[:, ::13][:, :128].reshape(-1, Dq)
    qp = (qi @ Wq_e + bq_e).reshape(Bq, -1, H, Dq // H)
    kp = (ki @ Wk_e + bk_e).reshape(Bq, -1, H, Dq // H)
    sc = np.einsum("bqhd,bkhd->bhqk", qp, kp) / np.sqrt(Dq // H)
    shift_c = float(sc.max() + 2.0 * sc.std() - math.log(200.0))

    scales = (s_wq, s_wk, s_wv, s_wo, s_w1, s_w2, shift_c)
    nc = _get_nc(Sq, Qtok, Dq, H, MLP, N_CORES, scales)

    shared = {
        "wq8": _swz_qk((Wq_e * s_wq).astype(NP_F8)),
        "wk8": _swz_qk((Wk_e * s_wk).astype(NP_F8)),
        "wv8": _swz_v((Wv_e * s_wv).astype(NP_F8)),
        "wo8": _swz_wo((Wo * s_wo).astype(NP_F8)),
        "w18": _swz_w1((W1_e * s_w1).astype(NP_F8)),
        "w28": _swz_w2(_split_f8(W2 * s_w2)),
        "bq": (bq_e * s_wq).astype(np.float32),
        "bk": (bk_e * s_wk).astype(np.float32),
        "bv16": (bv_e * s_wv).astype(ml_dtypes.bfloat16),
        "bo": bo.astype(np.float32),
        "b1": b1_e.astype(np.float32),
        "b2": b2.astype(np.float32),
        "ones32": np.ones((P, P), np.float32),
    }
    in_maps = []
    for c in range(N_CORES):
        b = c // 2
        half = c % 2
        xb = x[b]
        xr = np.concatenate(
            [xb[half * Qtok:(half + 1) * Qtok],
             xb[(1 - half) * Qtok:(2 - half) * Qtok]], axis=0)
        m = dict(shared)
        m["xT"] = np.ascontiguousarray(xr.T)
        in_maps.append(m)
    return nc, in_maps, Qtok


def unshard(res, Bq, Sq, Dq, Qtok):
    out = np.empty((Bq, Sq, Dq), np.float32)
    for c in range(N_CORES):
        b = c // 2
        half = c % 2
        out[b, half * Qtok:(half + 1) * Qtok, :] = res.results[c]["yT"].T
    return out


def kernel(**inputs):
    x = np.asarray(inputs["x"], np.float32)
    Bq, Sq, Dq = x.shape
    nc, in_maps, Qtok = prepare(inputs)
    res = run_bass_kernel_spmd(nc, in_maps, core_ids=list(range(N_CORES)))
    return unshard(res, Bq, Sq, Dq, Qtok)


# revision 30
# speedup vs baseline: 1.0840x; 1.0466x over previous
"""Trainium2 Bass kernel for a dense transformer block (LN1 -> MHA -> LN2 -> MLP).

Sharding: 8 cores = (batch b in 0..3) x (sequence half in 0..1), zero
cross-core communication. Each core's input tokens are reordered on the host
so its 1024 query tokens are always tokens 0..1023 of its 2048-token view
(key/value order is irrelevant to attention), letting one SPMD program serve
every core and the query-side LN reuse the full-sequence LN output.

Precision: fp8e4m3 DoubleRow matmuls for QKV/O projections, ctx, and the MLP
(weights pre-scaled by power-of-2 factors on the host; descales fold into
existing bias/scale stages, so they cost nothing). Scores stay bf16.
LayerNorm gain/bias are folded into the following weights on the host
(mathematically exact), so the device LN is a pure (x-mu)*rstd normalize.

Softmax: exp(score - C) with a host-estimated shift C keeping exp outputs in
fp8 range; the denominator is produced by a ones-column appended to V inside
the ctx DoubleRow matmul (out partition 65), so it costs no extra PE time.

Schedule: the Act engine's exp stream (16 blocks x ~16us) is the backbone;
everything else is emitted as "filler units" pulled into slots between the
per-kcp score/ctx groups of each attention block so the in-order PE queue
stays fed without ever delaying the next scores (which would starve Act).
Projections stream into the qq=0 attention blocks; wo/ln2/fc1 of qq=0 stream
into the qq=1 blocks. fc1 results are staged pre-gelu in bf16 (aliasing the
Y1 arena via bitcast) so gelus run as two contiguous islands (2 activation-
table switches instead of ~64); fc2(0) fills the late qq=1 blocks. Weights
are pre-swizzled on the host into the exact SBUF tile images so every weight
DMA is fully contiguous (2x descriptor throughput vs 128B strides).
"""

import math
import sys

if '/opt/trn_rl_repo' not in sys.path:
    sys.path.insert(0, '/opt/trn_rl_repo')

import numpy as np
import ml_dtypes

import concourse.tile as tile
import concourse.mybir as mybir
from concourse import bacc
from concourse.bass import ts
from concourse.bass_utils import run_bass_kernel_spmd

P = 128
F32 = mybir.dt.float32
F32R = mybir.dt.float32r
BF16 = mybir.dt.bfloat16
F8 = mybir.dt.float8e4
AF = mybir.ActivationFunctionType
DR = mybir.MatmulPerfMode.DoubleRow
ALU = mybir.AluOpType
EPS = 1e-6

B, S, D, H, MLP = 4, 2048, 1024, 16, 4096
N_CORES = 8
NP_F8 = ml_dtypes.float8_e4m3


def build_bass(T, Q, Dm, Hh, Mlp, n_cores, scales, dbg=False):
    s_wq, s_wk, s_wv, s_wo, s_w1, s_w2, shift_c = scales
    dh = Dm // Hh
    assert dh == 64
    n_dc = Dm // P          # 8 feature chunks
    n_cj = n_dc // 2        # 4 DoubleRow k-pair steps over D
    n_tk = T // P           # 16 token chunks
    TB = 512
    n_tb = T // TB          # 4
    KTB = 512               # K projection token slice
    n_ktb = T // KTB        # 4
    QQ = 512
    n_qq = Q // QQ          # 2
    n_mo = Mlp // P         # 32
    n_m2 = n_mo // 2        # 16 DoubleRow k-pair steps over MLP
    n_hp = Hh // 2          # 8 head pairs
    NO = 512
    inv_d = 1.0 / Dm
    exp_scale = 0.125 / (s_wq * s_wk)
    c_wo = 1.0 / (s_wo * s_wv)
    inv_s1 = 1.0 / s_w1
    inv_s2 = 1.0 / s_w2

    nc = bacc.Bacc("TRN2", target_bir_lowering=False, debug=False,
                   enable_asserts=False, num_devices=n_cores)

    def din(name, shape, dt):
        return nc.dram_tensor(name, shape, dt, kind="ExternalInput").ap()

    xT_d = din("xT", (Dm, T), F32)
    # host-swizzled weight images: per-partition-contiguous SBUF tile layouts
    wq_d = din("wq8", (P, n_dc * n_cj * 2 * P), F8)
    wk_d = din("wk8", (P, n_dc * n_cj * 2 * P), F8)
    wv_d = din("wv8", (P, 2 * n_cj * 2 * NO), F8)
    wo_d = din("wo8", (2, P, n_cj * 2 * 4 * P), F8)
    w1_d = din("w18", (n_mo // 4, P, n_cj * 2 * 4 * P), F8)
    w2_d = din("w28", (2, n_dc, P, n_m2 * 2 * P), F8)
    bq_d, bk_d = din("bq", (Dm,), F32), din("bk", (Dm,), F32)
    bv_d, bo_d = din("bv16", (Dm,), BF16), din("bo", (Dm,), F32)
    b1_d, b2_d = din("b1", (Mlp,), F32), din("b2", (Dm,), F32)
    ones_d = din("ones32", (P, P), F32)
    yT_d = nc.dram_tensor("yT", (Dm, Q), F32, kind="ExternalOutput").ap()

    with tile.TileContext(nc) as tc, \
         tc.tile_pool(name="const", bufs=1) as constp, \
         tc.tile_pool(name="p_res", bufs=1) as p_res, \
         tc.tile_pool(name="p_kv", bufs=1) as p_kv, \
         tc.tile_pool(name="p_exp", bufs=2) as p_exp, \
         tc.tile_pool(name="p_rb", bufs=1) as p_rb, \
         tc.tile_pool(name="p_ct", bufs=1) as p_ct, \
         tc.tile_pool(name="p_wos", bufs=2) as p_wos, \
         tc.tile_pool(name="p_w1", bufs=2) as p_w1, \
         tc.tile_pool(name="p_out", bufs=1) as p_out, \
         tc.tile_pool(name="ps_sc", bufs=2, space="PSUM") as ps_sc, \
         tc.tile_pool(name="ps_ctx", bufs=2, space="PSUM") as ps_ctx, \
         tc.tile_pool(name="ps_sh", bufs=2, space="PSUM") as ps_sh:

        ones_fr = constp.tile([P, P], F32R)
        nc.sync.dma_start(ones_fr[:], ones_d[:, :].bitcast(F32R))
        ones_f = constp.tile([P, P], BF16)
        nc.vector.memset(ones_f[:], 1.0)
        negc_t = constp.tile([P, 1], F32)
        nc.vector.memset(negc_t[:], -shift_c)

        def vec_tile(src, n, nm):
            t = constp.tile([P, n], F32, tag=nm, name=nm)
            nc.sync.dma_start(t[:], src.rearrange("(c p) -> p c", p=P))
            return t

        bq_t, bk_t = vec_tile(bq_d, n_dc, "bq"), vec_tile(bk_d, n_dc, "bk")
        bo_t, b2_t = vec_tile(bo_d, n_dc, "bo"), vec_tile(b2_d, n_dc, "b2")
        b1_t = vec_tile(b1_d, n_mo, "b1")

        XQ = p_res.tile([P, n_dc, Q], BF16)       # residual stream (bf16)
        KT = p_kv.tile([P, n_dc, T], BF16)
        QT = p_kv.tile([P, n_dc, Q], BF16)
        VT = p_kv.tile([P, n_tk, Hh, 65], F8)
        nc.gpsimd.memset(VT[:, :, :, 64:65], 1.0)
        CT8 = p_ct.tile([P, n_dc, Q], F8)

        def attn_block(qq, hp, pull=None, fsc=1, fctx=1):
            """One head-pair of attention for query chunk qq, software-
            pipelined, with filler slots after each score/ctx group."""
            qsl = ts(qq, QQ)
            exps = [p_exp.tile([P, n_tk, QQ], F8, tag="expT", name="expT")
                    for _ in range(2)]
            pcs = [ps_ctx.tile([65, QQ], F32, tag="ps_c", name="ps_c")
                   for _ in range(2)]
            nk2 = n_tk // 2
            LAG = 2
            for j in range(nk2 + LAG):
                if j < nk2:
                    for hi in range(2):
                        r0 = hi * 64
                        pss = ps_sc.tile([P, 2, QQ], F32, tag="ps_s",
                                         name="ps_s")
                        for jj in range(2):
                            nc.tensor.matmul(
                                pss[:, jj, :],
                                KT[r0:r0 + 64, hp, ts(2 * j + jj, P)],
                                QT[r0:r0 + 64, hp, qsl],
                                start=True, stop=True)
                        nc.scalar.activation(
                            exps[hi][:, 2 * j:2 * j + 2, :],
                            pss[:, :, :], AF.Exp,
                            scale=exp_scale, bias=negc_t[:, 0:1])
                    if pull:
                        pull(fsc)
                if j >= LAG:
                    kcp = j - LAG
                    for hi in range(2):
                        h = 2 * hp + hi
                        nc.tensor.matmul(
                            pcs[hi][:, :],
                            VT[:, 2 * kcp:2 * kcp + 2, h, 0:65],
                            exps[hi][:, 2 * kcp:2 * kcp + 2, :],
                            start=(kcp == 0), stop=(kcp == nk2 - 1),
                            perf_mode=DR)
                    if pull:
                        pull(fctx)
            for hi in range(2):
                r0 = hi * 64
                rb1 = p_rb.tile([1, QQ], F32, tag="rb1")
                nc.vector.reciprocal(rb1[:], pcs[hi][64:65, :])
                rbb = p_rb.tile([64, QQ], F32, tag="rbb")
                nc.gpsimd.partition_broadcast(rbb[:], rb1[:])
                nc.vector.tensor_mul(CT8[r0:r0 + 64, hp, qsl],
                                     pcs[hi][0:64, :], rbb[:])

        def make_pull(gen):
            done = [False]
            def pull(n=1):
                if done[0]:
                    return
                for _ in range(n):
                    try:
                        next(gen)
                    except StopIteration:
                        done[0] = True
                        return
            return pull

        w1_map = {}

        def w1_need(g):
            if g not in w1_map:
                wt = p_w1.tile([P, n_cj, 2, 4, P], F8, tag="w1")
                nc.sync.dma_start(
                    wt[:].rearrange("p a b c d -> p (a b c d)"), w1_d[g])
                w1_map[g] = wt
            return w1_map[g]

        # ================= Phase A: LN1 + projections + attn(qq=0) =========
        with tc.tile_pool(name="p_xn8", bufs=1) as p_xn8:

            XN8 = p_xn8.tile([P, n_dc, T], F8)
            bv_row = p_xn8.tile([1, Dm], BF16)
            nc.sync.dma_start(bv_row[:, :], bv_d[None, :])
            bv_bc = p_xn8.tile([P, Dm], BF16)
            nc.gpsimd.partition_broadcast(bv_bc[:], bv_row[:])

            # ---- LN1 (its x staging pools close before the weight pools
            # open, keeping peak SBUF in budget) ----
            with tc.tile_pool(name="p_x", bufs=2) as p_x, \
                 tc.tile_pool(name="p_t", bufs=2) as p_t:
                for tb in range(n_tb):
                    tsl = ts(tb, TB)
                    xc = p_x.tile([P, n_dc, TB], F32R, tag="xc")
                    for dc in range(n_dc):
                        nc.sync.dma_start(xc[:, dc, :],
                                          xT_d[ts(dc, P), tsl].bitcast(F32R))
                    st = ps_sc.tile([P, 2, TB], F32, tag="ps_s", name="ps_s")
                    for dc in range(n_dc):
                        nc.tensor.matmul(st[:, 0, :], ones_fr[:],
                                         xc[:, dc, :],
                                         start=(dc == 0),
                                         stop=(dc == n_dc - 1))
                        xsq = p_t.tile([P, TB], BF16, tag="xsq")
                        nc.scalar.activation(xsq[:],
                                             xc[:, dc, :].bitcast(F32),
                                             AF.Square)
                        nc.tensor.matmul(st[:, 1, :], ones_f[:], xsq[:],
                                         start=(dc == 0),
                                         stop=(dc == n_dc - 1))
                        if tb < Q // TB:
                            nc.vector.tensor_copy(XQ[:, dc, tsl],
                                                  xc[:, dc, :].bitcast(F32))
                    mbc = p_t.tile([P, TB], F32, tag="mbc")
                    nc.vector.tensor_scalar_mul(mbc[:], st[:, 0, :], inv_d)
                    var = p_t.tile([P, TB], F32, tag="var")
                    nc.vector.tensor_scalar(var[:], st[:, 1, :], inv_d, EPS,
                                            op0=ALU.mult, op1=ALU.add)
                    m2 = p_t.tile([P, TB], F32, tag="tn")
                    nc.vector.tensor_mul(m2[:], mbc[:], mbc[:])
                    nc.vector.tensor_sub(var[:], var[:], m2[:])
                    nc.scalar.activation(var[:], var[:], AF.Sqrt)
                    rstd = p_t.tile([P, TB], F32, tag="rstd")
                    nc.vector.reciprocal(rstd[:], var[:])
                    for dc in range(n_dc):
                        t0 = p_t.tile([P, TB], F32, tag="tn")
                        nc.vector.tensor_sub(t0[:],
                                             xc[:, dc, :].bitcast(F32),
                                             mbc[:])
                        nc.gpsimd.tensor_mul(XN8[:, dc, tsl], t0[:],
                                             rstd[:])

            with tc.tile_pool(name="p_wk", bufs=1) as p_wk:
                wk_t = p_wk.tile([P, n_dc, n_cj, 2, P], F8, tag="wk")
                nc.sync.dma_start(
                    wk_t[:].rearrange("p a b c d -> p (a b c d)"), wk_d[:, :])
                wq_t = p_wk.tile([P, n_dc, n_cj, 2, P], F8, tag="wq")
                nc.sync.dma_start(
                    wq_t[:].rearrange("p a b c d -> p (a b c d)"), wq_d[:, :])
                wv_t = p_wk.tile([P, 2, n_cj, 2, NO], F8, tag="wv")
                nc.sync.dma_start(
                    wv_t[:].rearrange("p a b c d -> p (a b c d)"), wv_d[:, :])

                def k_unit(mo, tb):
                    tsl = ts(tb, KTB)
                    ps = ps_sh.tile([P, KTB], F32, tag="sh")
                    for cj in range(n_cj):
                        nc.tensor.matmul(
                            ps[:], wk_t[:, mo, cj, :, :],
                            XN8[:, 2 * cj:2 * cj + 2, tsl],
                            start=(cj == 0), stop=(cj == n_cj - 1),
                            perf_mode=DR)
                    nc.vector.tensor_scalar_add(KT[:, mo, tsl], ps[:],
                                                bk_t[:, mo:mo + 1])

                def q_unit(mo, qb):
                    qsl = ts(qb, QQ)
                    ps = ps_sh.tile([P, QQ], F32, tag="sh")
                    for cj in range(n_cj):
                        nc.tensor.matmul(
                            ps[:], wq_t[:, mo, cj, :, :],
                            XN8[:, 2 * cj:2 * cj + 2, qsl],
                            start=(cj == 0), stop=(cj == n_cj - 1),
                            perf_mode=DR)
                    nc.vector.tensor_scalar_add(QT[:, mo, qsl], ps[:],
                                                bq_t[:, mo:mo + 1])

                def v_unit(no, to):
                    ps = ps_sh.tile([P, NO], F32, tag="sh")
                    for cj in range(n_cj):
                        nc.tensor.matmul(
                            ps[:], XN8[:, 2 * cj:2 * cj + 2, ts(to, P)],
                            wv_t[:, no, cj, :, :],
                            start=(cj == 0), stop=(cj == n_cj - 1),
                            perf_mode=DR)
                    nc.vector.tensor_add(VT[:, to, 8 * no:8 * no + 8, 0:64],
                                         ps[:], bv_bc[:, ts(no, NO)])

                # minimal pre-attention work: K/Q/V for the first blocks
                for tb in range(n_ktb):
                    k_unit(0, tb)
                q_unit(0, 0)
                for to in range(4):
                    v_unit(0, to)

                def projection_gen():
                    for to in range(4, 16):
                        v_unit(0, to)
                        yield
                    for mo in range(1, n_dc):
                        for tb in range(n_ktb):
                            k_unit(mo, tb)
                            yield
                        q_unit(mo, 0)
                        yield
                        if mo in (4, 5):
                            for to in range(8 * (mo - 4), 8 * (mo - 3)):
                                v_unit(1, to)
                                yield
                    for mo in range(n_dc):
                        q_unit(mo, 1)
                        yield
                    # prefetch wo groups + w1 group 0 for phase B
                    for g in range(2):
                        wt = p_wos.tile([P, n_cj, 2, 4, P], F8, tag="wo")
                        nc.sync.dma_start(
                            wt[:].rearrange("p a b c d -> p (a b c d)"),
                            wo_d[g])
                        wo_tiles.append(wt)
                    yield
                    w1_need(0)
                    yield

                wo_tiles = []
                gA = projection_gen()
                pullA = make_pull(gA)
                for hp in range(n_hp):
                    attn_block(0, hp, pull=pullA, fsc=1, fctx=0)
                for _ in gA:
                    pass

        # ================= Phase B: attn(qq=1) + MLP =======================
        with tc.tile_pool(name="p_mlp", bufs=1) as p_mlp, \
             tc.tile_pool(name="p_t2", bufs=2) as p_t2, \
             tc.tile_pool(name="p_t2s", bufs=1) as p_t2s, \
             tc.tile_pool(name="p_w2", bufs=2) as p_w2, \
             tc.tile_pool(name="p_y1", bufs=1) as p_y1:

            XN2 = p_mlp.tile([P, n_dc, 2, Q], F8)  # [.., hi/lo, ..]
            y1s = {}

            def wo_unit(qq, mo):
                qsl = ts(qq, QQ)
                ps = ps_sh.tile([P, QQ], F32, tag="sh")
                for cj in range(n_cj):
                    nc.tensor.matmul(
                        ps[:], wo_tiles[mo // 4][:, cj, :, mo % 4, :],
                        CT8[:, 2 * cj:2 * cj + 2, qsl],
                        start=(cj == 0), stop=(cj == n_cj - 1),
                        perf_mode=DR)
                tw = p_out.tile([P, QQ], F32, tag="ot")
                nc.vector.tensor_scalar(tw[:], ps[:], c_wo,
                                        bo_t[:, mo:mo + 1],
                                        op0=ALU.mult, op1=ALU.add)
                nc.vector.tensor_add(XQ[:, mo, qsl], tw[:], XQ[:, mo, qsl])

            def ln2_block(qq):
                qsl = ts(qq, QQ)
                st2 = ps_sc.tile([P, 2, QQ], F32, tag="ps_s", name="ps_s")
                for dc in range(n_dc):
                    nc.tensor.matmul(st2[:, 0, :], ones_f[:], XQ[:, dc, qsl],
                                     start=(dc == 0), stop=(dc == n_dc - 1))
                    sq = p_t2.tile([P, QQ], BF16, tag="sq2")
                    nc.gpsimd.tensor_mul(sq[:], XQ[:, dc, qsl],
                                         XQ[:, dc, qsl])
                    nc.tensor.matmul(st2[:, 1, :], ones_f[:], sq[:],
                                     start=(dc == 0), stop=(dc == n_dc - 1))
                mbc = p_t2s.tile([P, QQ], F32, tag="mbc2")
                nc.vector.tensor_scalar_mul(mbc[:], st2[:, 0, :], inv_d)
                var = p_t2s.tile([P, QQ], F32, tag="var2")
                nc.vector.tensor_scalar(var[:], st2[:, 1, :], inv_d, EPS,
                                        op0=ALU.mult, op1=ALU.add)
                m2 = p_t2.tile([P, QQ], F32, tag="tn2")
                nc.vector.tensor_mul(m2[:], mbc[:], mbc[:])
                nc.vector.tensor_sub(var[:], var[:], m2[:])
                # rstd = rsqrt(var) on DVE only: seed from 1/var + Newton
                r = p_t2s.tile([P, QQ], F32, tag="rstd2")
                nc.vector.reciprocal_approx_fast(r[:], var[:])
                nc.vector.tensor_scalar(r[:], r[:], 0.72, 0.35,
                                        op0=ALU.mult, op1=ALU.add)
                for _ in range(3):
                    t1 = p_t2.tile([P, QQ], F32, tag="tn2")
                    nc.vector.tensor_mul(t1[:], r[:], r[:])
                    nc.vector.tensor_mul(t1[:], t1[:], var[:])
                    nc.vector.tensor_scalar(t1[:], t1[:], -0.5, 1.5,
                                            op0=ALU.mult, op1=ALU.add)
                    nc.vector.tensor_mul(r[:], r[:], t1[:])
                for dc in range(n_dc):
                    t0 = p_t2.tile([P, QQ], F32, tag="tn2")
                    nc.gpsimd.tensor_sub(t0[:], XQ[:, dc, qsl], mbc[:])
                    m = p_t2.tile([P, QQ], F32, tag="m32")
                    nc.vector.tensor_mul(m[:], t0[:], r[:])
                    nc.vector.tensor_copy(XN2[:, dc, 0, qsl], m[:])
                    nc.gpsimd.tensor_sub(XN2[:, dc, 1, qsl], m[:],
                                         XN2[:, dc, 0, qsl])

            def y1_tile(qq):
                if qq not in y1s:
                    y1s[qq] = p_y1.tile([P, n_mo, 2, QQ], F8, tag="y1",
                                        name="y1")
                return y1s[qq]

            def z1_view(Y1, mo):
                # bf16 view of Y1[:, mo, :, :]'s bytes (pre-gelu staging)
                return Y1[:, mo, :, :].bitcast(BF16).rearrange(
                    "p a b -> p (a b)")

            def fc1_mm_unit(qq, mo, staged):
                qsl = ts(qq, QQ)
                Y1 = y1_tile(qq)
                wt = w1_need(mo // 4)
                if mo % 4 == 0 and (mo // 4) + 1 < n_mo // 4:
                    w1_need((mo // 4) + 1)   # prefetch next group
                ps = ps_sh.tile([P, QQ], F32, tag="sh")
                for lv in range(2):
                    for cj in range(n_cj):
                        nc.tensor.matmul(
                            ps[:], wt[:, cj, :, mo % 4, :],
                            XN2[:, 2 * cj:2 * cj + 2, lv, qsl],
                            start=(lv == 0 and cj == 0),
                            stop=(lv == 1 and cj == n_cj - 1),
                            perf_mode=DR)
                if staged:
                    nc.gpsimd.tensor_copy(z1_view(Y1, mo), ps[:])
                    return None
                return ps

            def gelu_island(qq, mo0, mo1):
                Y1 = y1_tile(qq)
                for mo in range(mo0, mo1):
                    g32 = p_t2.tile([P, QQ], F32, tag="m32")
                    nc.scalar.activation(g32[:], z1_view(Y1, mo), AF.Gelu,
                                         bias=b1_t[:, mo:mo + 1],
                                         scale=inv_s1)
                    nc.vector.tensor_copy(Y1[:, mo, 0, :], g32[:])
                    nc.gpsimd.tensor_sub(Y1[:, mo, 1, :], g32[:],
                                         Y1[:, mo, 0, :])
                # fence: rewrite negc_t (same value) from the island's last
                # split so every later exp waits for the whole island --
                # keeps the gelu run contiguous (one act-table switch each
                # way) under the readiness-greedy tile scheduler
                nc.vector.tensor_scalar(negc_t[:, 0:1],
                                        Y1[:, mo1 - 1, 1, 0:1], 0.0,
                                        -shift_c, op0=ALU.mult, op1=ALU.add)

            def fc1_plain_unit(qq, mo):
                # tail variant: inline gelu (gelus end up consecutive)
                Y1 = y1_tile(qq)
                ps = fc1_mm_unit(qq, mo, staged=False)
                g32 = p_t2.tile([P, QQ], F32, tag="m32")
                nc.scalar.activation(g32[:], ps[:], AF.Gelu,
                                     bias=b1_t[:, mo:mo + 1],
                                     scale=inv_s1)
                nc.vector.tensor_copy(Y1[:, mo, 0, :], g32[:])
                nc.gpsimd.tensor_sub(Y1[:, mo, 1, :], g32[:],
                                     Y1[:, mo, 0, :])

            def fc2_gen(qq, m0=0, m1=None):
                qsl = ts(qq, QQ)
                if m1 is None:
                    m1 = n_dc
                Y1 = y1s[qq]
                if m1 == n_dc:
                    y1s.pop(qq)
                for mo2 in range(m0, m1):
                    wth = p_w2.tile([P, n_m2, 2, P], F8, tag="w2h")
                    nc.sync.dma_start(
                        wth[:].rearrange("p a b c -> p (a b c)"),
                        w2_d[0, mo2])
                    wtl = p_w2.tile([P, n_m2, 2, P], F8, tag="w2l")
                    nc.sync.dma_start(
                        wtl[:].rearrange("p a b c -> p (a b c)"),
                        w2_d[1, mo2])
                    ps = ps_sh.tile([P, QQ], F32, tag="sh")
                    terms = [(wth, 0), (wth, 1), (wtl, 0)]
                    for ti, (wt, lv) in enumerate(terms):
                        for cj in range(n_m2):
                            nc.tensor.matmul(
                                ps[:], wt[:, cj, :, :],
                                Y1[:, 2 * cj:2 * cj + 2, lv, :],
                                start=(ti == 0 and cj == 0),
                                stop=(ti == 2 and cj == n_m2 - 1),
                                perf_mode=DR)
                            if cj == n_m2 // 2 - 1:
                                yield
                        yield
                    ot = p_out.tile([P, QQ], F32, tag="ot")
                    nc.vector.tensor_scalar(ot[:], ps[:], inv_s2,
                                            b2_t[:, mo2:mo2 + 1],
                                            op0=ALU.mult, op1=ALU.add)
                    nc.vector.tensor_add(ot[:], ot[:], XQ[:, mo2, qsl])
                    nc.sync.dma_start(yT_d[ts(mo2, P), qsl], ot[:])

            def mlp0_gen():
                # stretch fillers for qq=0 MLP, pulled into attn(1,*) slots
                for mo in range(n_dc):
                    wo_unit(0, mo)
                    yield
                ln2_block(0)
                # bubbles: let the DVE/Pool XN2 chain finish before the
                # first fc1 matmul hits the in-order PE queue
                for _ in range(12):
                    yield
                for mo in range(16):
                    fc1_mm_unit(0, mo, staged=True)
                    yield
                gelu_island(0, 0, 16)      # ~11us Act island
                for mo in range(16, n_mo):
                    fc1_mm_unit(0, mo, staged=True)
                    yield
                gelu_island(0, 16, n_mo)   # second island
                yield
                yield
                yield from fc2_gen(0, 0, 4)

            g0 = mlp0_gen()
            pull0 = make_pull(g0)
            for hp in range(n_hp):
                attn_block(1, hp, pull=pull0, fsc=1,
                           fctx=(1 if hp >= 5 else 0))
            for _ in g0:
                pass

            # ---------------- tail: qq=1 MLP ------------------------------
            w1_map.clear()
            for mo in range(n_dc):
                wo_unit(1, mo)
            ln2_block(1)
            for _ in fc2_gen(0, 4, 8):   # overlaps the XN2(1) latency chain
                pass
            w1_need(0)
            for mo in range(n_mo):
                fc1_plain_unit(1, mo)
            for _ in fc2_gen(1):
                pass
    nc.compile()
    return nc


_NC_CACHE = {}


def _get_nc(T, Q, Dm, Hh, Mlp, n_cores,
            scales=(16.0, 16.0, 16.0, 16.0, 16.0, 16.0, 3.5)):
    key = (T, Q, Dm, Hh, Mlp, n_cores, tuple(scales))
    if key not in _NC_CACHE:
        _NC_CACHE[key] = build_bass(T, Q, Dm, Hh, Mlp, n_cores, scales)
    return _NC_CACHE[key]


def _split_f8(w):
    hi = w.astype(NP_F8)
    lo = (w - hi.astype(np.float32)).astype(NP_F8)
    return np.stack([hi, lo])


def _pow2_scale(absmax, target=128.0):
    a = float(absmax)
    if not np.isfinite(a) or a <= 0:
        return 1.0
    return float(2.0 ** math.floor(math.log2(target / a)))


def _swz_qk(w8):
    # (D, D) -> [p][mo][cj][j][m] SBUF image, flattened to (P, 8192)
    t = w8.reshape(4, 2, P, 8, P)           # (c, j, p, mo, m)
    return np.ascontiguousarray(t.transpose(2, 3, 0, 1, 4)).reshape(P, -1)


def _swz_v(w8):
    t = w8.reshape(4, 2, P, 2, 512)         # (c, j, p, no, m)
    return np.ascontiguousarray(t.transpose(2, 3, 0, 1, 4)).reshape(P, -1)


def _swz_wo(w8):
    t = w8.reshape(4, 2, P, 2, 4, P)        # (c, j, p, g, mo, m)
    return np.ascontiguousarray(t.transpose(3, 2, 0, 1, 4, 5)).reshape(
        2, P, -1)


def _swz_w1(w8):
    t = w8.reshape(4, 2, P, 8, 4, P)        # (c, j, p, g, mo, m)
    return np.ascontiguousarray(t.transpose(3, 2, 0, 1, 4, 5)).reshape(
        8, P, -1)


def _swz_w2(w8_2):
    # (2, MLP, D) -> (2, 8, P, 4096): [s][mo2][p][c][j][m]
    t = w8_2.reshape(2, 16, 2, P, 8, P)     # (s, c, j, p, mo2, m)
    return np.ascontiguousarray(t.transpose(0, 4, 3, 1, 2, 5)).reshape(
        2, 8, P, -1)


def prepare(inputs):
    """Host-side prep: LN folding, fp8 quantization, per-core input maps."""
    f = lambda k: np.asarray(inputs[k], np.float32)
    x = f("x")
    Bq, Sq, Dq = x.shape
    Qtok = Sq // 2
    g1, b1ln = f("ln1_g"), f("ln1_b")
    g2, b2ln = f("ln2_g"), f("ln2_b")
    Wq, Wk, Wv, Wo = f("Wq"), f("Wk"), f("Wv"), f("Wo")
    W1, W2 = f("W1"), f("W2")
    bq, bk, bv, bo = f("bq"), f("bk"), f("bv"), f("bo")
    b1, b2 = f("b1"), f("b2")

    # fold LN1 gain/bias into QKV, LN2 gain/bias into W1 (exact)
    Wq_e = g1[:, None] * Wq
    Wk_e = g1[:, None] * Wk
    Wv_e = g1[:, None] * Wv
    bq_e = bq + b1ln @ Wq
    bk_e = bk + b1ln @ Wk
    bv_e = bv + b1ln @ Wv
    W1_e = g2[:, None] * W1
    b1_e = b1 + b2ln @ W1

    s_wq = _pow2_scale(np.abs(Wq_e).max())
    s_wk = _pow2_scale(np.abs(Wk_e).max())
    # V result is stored in fp8 still scaled by s_wv: bound both weight and
    # activation range (sigma of v_j ~ col norm of Wv_e, x is LN'd)
    vcol = np.sqrt((Wv_e ** 2).sum(0))
    vmag = max(float((vcol * 8).max()), float(np.abs(bv_e).max() * 4), 1e-6)
    s_wv = min(_pow2_scale(np.abs(Wv_e).max()),
               _pow2_scale(vmag, target=200.0))
    s_wo = _pow2_scale(np.abs(Wo).max())
    s_w1 = _pow2_scale(np.abs(W1_e).max())
    s_w2 = _pow2_scale(np.abs(W2).max())

    # estimate max attention score for the exp shift C (sampled)
    mu = x.mean(-1, keepdims=True)
    va = x.var(-1, keepdims=True)
    xn_h = (x - mu) / np.sqrt(va + EPS)
    qi = xn_h[:, ::89][:, :16].reshape(-1, Dq)
    ki = xn_h[:, ::13][:, :128].reshape(-1, Dq)
    qp = (qi @ Wq_e + bq_e).reshape(Bq, -1, H, Dq // H)
    kp = (ki @ Wk_e + bk_e).reshape(Bq, -1, H, Dq // H)
    sc = np.einsum("bqhd,bkhd->bhqk", qp, kp) / np.sqrt(Dq // H)
    shift_c = float(sc.max() + 2.0 * sc.std() - math.log(200.0))

    scales = (s_wq, s_wk, s_wv, s_wo, s_w1, s_w2, shift_c)
    nc = _get_nc(Sq, Qtok, Dq, H, MLP, N_CORES, scales)

    shared = {
        "wq8": _swz_qk((Wq_e * s_wq).astype(NP_F8)),
        "wk8": _swz_qk((Wk_e * s_wk).astype(NP_F8)),
        "wv8": _swz_v((Wv_e * s_wv).astype(NP_F8)),
        "wo8": _swz_wo((Wo * s_wo).astype(NP_F8)),
        "w18": _swz_w1((W1_e * s_w1).astype(NP_F8)),
        "w28": _swz_w2(_split_f8(W2 * s_w2)),
        "bq": (bq_e * s_wq).astype(np.float32),
        "bk": (bk_e * s_wk).astype(np.float32),
        "bv16": (bv_e * s_wv).astype(ml_dtypes.bfloat16),
        "bo": bo.astype(np.float32),
        "b1": b1_e.astype(np.float32),
        "b2": b2.astype(np.float32),
        "ones32": np.ones((P, P), np.float32),
    }
    in_maps = []
    for c in range(N_CORES):
        b = c // 2
        half = c % 2
        xb = x[b]
        xr = np.concatenate(
            [xb[half * Qtok:(half + 1) * Qtok],
             xb[(1 - half) * Qtok:(2 - half) * Qtok]], axis=0)
        m = dict(shared)
        m["xT"] = np.ascontiguousarray(xr.T)
        in_maps.append(m)
    return nc, in_maps, Qtok


def unshard(res, Bq, Sq, Dq, Qtok):
    out = np.empty((Bq, Sq, Dq), np.float32)
    for c in range(N_CORES):
        b = c // 2
        half = c % 2
        out[b, half * Qtok:(half + 1) * Qtok, :] = res.results[c]["yT"].T
    return out


def kernel(**inputs):
    x = np.asarray(inputs["x"], np.float32)
    Bq, Sq, Dq = x.shape
    nc, in_maps, Qtok = prepare(inputs)
    res = run_bass_kernel_spmd(nc, in_maps, core_ids=list(range(N_CORES)))
    return unshard(res, Bq, Sq, Dq, Qtok)
